# revision 15
# baseline (speedup 1.0000x reference)
"""Trainium2 Bass kernel for nn_MOTASG_KO_Reg (ragged graph-conv KO regression).

Strategy (8 NeuronCores, data-parallel over node rows):
  - N=16384 nodes = 16 batch samples x 1024 entities. Core c owns rows
    [2048c, 2048c+2048) = batch samples 2c, 2c+1.
  - Activations kept feature-major ("transposed", [feat, rows]) on chip so
    every linear is a native PE matmul (fp16 operands, fp32 PSUM).
  - name/desc path computed once on 128 entities/core, AllGathered, folded
    into cross INSIDE the fus PSUM chain via identity-inject matmuls.
  - z is never materialized: m2 = z @ enc_W is expanded as
      m2 = x_c @ enc_W + pre_c @ (pre_W @ enc_W) + lrelu(u) @ enc_W
    with pre_W @ enc_W precomputed on host. The dense part (m2d) runs
    during the fp8 AllGather window; the u part joins per tile after the
    gconv1 scatter.
  - gconv1 segment-sum via dma_gather + one-hot scatter matmuls in PSUM.
    Edges whose dst never feeds gconv2 are pruned (exact). Local-source
    edges (plus the self term as synthetic r->r edges) gather from fp16
    ag1_in DURING the AllGather; remote edges gather fp8 rows from the
    fp8 AllGather output.
  - All leaky-relus run on the scalar engine (ACTF.Prelu, PSUM input,
    per-partition bias AP, alpha=slope) - one op instead of three
    vector ops. Transposes run in fp16 (4x fp32 PE rate).
  - gconv2 source-side partials into the 1024 KO slots; partials are
    PE-transposed to [feat, slot] BEFORE the fp16 ReduceScatter so the
    readout (gate + softmax + weighted sum + regression) starts directly
    from rs_out with no transpose chain.
"""

import functools
import numpy as np

import concourse.bacc as bacc
import concourse.mybir as mybir
import concourse.tile as tile
from concourse import bass
from concourse.bass_utils import run_bass_kernel_spmd
from concourse.masks import make_identity

NE, B, KO = 1024, 16, 64
TX, OM, D = 768, 511, 512
N = NE * B
NCORE = 8
R = N // NCORE        # 2048 rows per core
NT = R // 128         # 16 row tiles per core
SLOPE = 0.3
F32 = mybir.dt.float32
F16 = mybir.dt.float16
F8 = mybir.dt.float8e4
I16 = mybir.dt.int16
AX = mybir.AxisListType.X
ALU = mybir.AluOpType
ACTF = mybir.ActivationFunctionType

WAVE = 8  # max gather chunks per dma_gather call
WCOLS = WAVE * 8


def _wave_sizes(C):
    """Two 4-chunk lead waves cut first-data latency; 8-chunk steady state."""
    return [4, 4] + [8] * ((C - 8) // 8)
DEBUG = False
TRACE = False
TRACE_KW = None


# ---------------------------------------------------------------------------
# host-side edge preparation
# ---------------------------------------------------------------------------

def _chunk_edges_per_tile(src, dstl, nch_t):
    """Sort (src->dst_local) into per-destination-tile 128-edge chunks."""
    C = sum(nch_t)
    idx = np.zeros((C, 128), np.int16)
    dstv = np.full((C, 128), -2.0, np.float32)
    t_of = dstl >> 7
    base = 0
    for t, nch in enumerate(nch_t):
        m = t_of == t
        s = src[m]
        d = (dstl[m] - (t << 7)).astype(np.float32)
        n = len(s)
        assert n <= nch * 128, (n, nch)
        full, rem = divmod(n, 128)
        for j in range(full):
            idx[base + j] = s[j * 128:(j + 1) * 128]
            dstv[base + j] = d[j * 128:(j + 1) * 128]
        if rem:
            idx[base + full, :rem] = s[full * 128:]
            dstv[base + full, :rem] = d[full * 128:]
        base += nch
    return idx, dstv


def _wrap_idx_waves(idx_chunks):
    """[C,128] int16 -> [128, C*8] wrapped per dma_gather call."""
    C = idx_chunks.shape[0]
    cols = []
    cur = 0
    for s in _wave_sizes(C):
        lin = idx_chunks[cur:cur + s].reshape(-1)
        cur += s
        cols.append(np.tile(lin.reshape(-1, 16).T, (8, 1)))
    return np.ascontiguousarray(np.concatenate(cols, axis=1))


def _sel_from_dstv(dstv, dt):
    C = dstv.shape[0]
    sel = (dstv[:, :, None] == np.arange(128, dtype=np.float32)[None, None, :])
    sel = sel.astype(dt)          # [C, 128 slot, 128 dst]
    return np.ascontiguousarray(sel.transpose(1, 0, 2).reshape(128, C * 128))


def _pad_w(w, rows, cols):
    out = np.zeros((rows, cols), np.float32)
    out[:w.shape[0], :w.shape[1]] = w
    return out


# ---------------------------------------------------------------------------
# program builder
# ---------------------------------------------------------------------------

@functools.lru_cache(maxsize=4)
def _build(nch1l_t, nch1r_t, nch2a_t, nch2b_t):
    """gconv1 chunks per dst tile split into local-src (gathered from ag1_in
    during AG1) and remote-src sets; gconv2 chunks per slot tile split by
    m2-row half so its gathers overlap m2 production. Totals are multiples
    of WAVE."""
    C1L = sum(nch1l_t)
    C1R = sum(nch1r_t)
    C2A = sum(nch2a_t)
    C2B = sum(nch2b_t)
    nc = bacc.Bacc("TRN2", num_swdge_queues=4)

    def din(name, shape, dtype=F16):
        return nc.dram_tensor(name, shape, dtype, kind="ExternalInput")

    x_t = din("x_t", [512, R])                  # [x | ko]^T fp16
    pre_t_d = din("pre_t", [512, R])
    ndemb = din("ndemb", [128, 12 * 128])
    # pre-transposed on host: [p, mo, ki, m] = W[ki*128+p, mo*128+m]
    name_W = din("name_W", [128, 6 * TX])
    desc_W = din("desc_W", [128, 6 * TX])
    omic_W = din("omic_W", [512, 512])
    fus_nd = din("fus_nd", [128, 4 * 12 * 128])
    fus_om = din("fus_om", [512, 512])
    ienc_W = din("ienc_W", [512, 512])
    wpe_W = din("wpe_W", [512, 512])            # pre_W @ enc_W (host)
    enc_W = din("enc_W", [512, 512])
    gate_W1 = din("gate_W1", [512, 512], F16)
    gw2reg = din("gw2reg", [128, 8], F16)
    bias_pf = din("bias_pf", [128, 26], F32)
    bias_rows = din("bias_rows", [96, 512], F16)
    idx1l_d = din("idx1l", [128, (C1L // WAVE) * WCOLS], I16)
    sel1l_d = din("sel1l", [128, C1L * 128], F16)
    idx1r_d = din("idx1r", [128, (C1R // WAVE) * WCOLS], I16)
    sel1r_d = din("sel1r", [128, C1R * 128], F8)
    idx2a_d = din("idx2a", [128, (C2A // WAVE) * WCOLS], I16)
    sel2a_d = din("sel2a", [128, C2A * 128], F16)
    idx2b_d = din("idx2b", [128, (C2B // WAVE) * WCOLS], I16)
    sel2b_d = din("sel2b", [128, C2B * 128], F16)
    out_d = nc.dram_tensor("out", [1, 2], F32, kind="ExternalOutput")

    agnd_in = nc.dram_tensor("agnd_in", [512, 128], F16)
    agnd_out = nc.dram_tensor("agnd_out", [NCORE * 512, 128], F16, addr_space="Shared")
    ag1_in = nc.dram_tensor("ag1_in", [R, 512], F16)
    ag8_in = nc.dram_tensor("ag8_in", [R // 4, 4 * 512], F8)
    ag8_out = nc.dram_tensor("ag8_out", [N // 4, 4 * 512], F8, addr_space="Shared")
    m2_a = nc.dram_tensor("m2_a", [12 * 128, 512], F16)
    m2_b = nc.dram_tensor("m2_b", [4 * 128, 512], F16)
    rs_in = nc.dram_tensor("rs_in", [NCORE * 512, 128], F16)
    rs_out = nc.dram_tensor("rs_out", [512, 128], F16)
    RG = [list(range(NCORE))]

    if DEBUG:
        dbg_cross = nc.dram_tensor("dbg_cross", [512, R], F16, kind="ExternalOutput")
        dbg_m2 = nc.dram_tensor("dbg_m2", [R, 512], F16, kind="ExternalOutput")
        dbg_zk = nc.dram_tensor("dbg_zk", [128, 512], F32, kind="ExternalOutput")

    with tile.TileContext(nc) as tc:
        with (
            tc.tile_pool(name="pbig", bufs=8) as pbig,
            tc.tile_pool(name="ppre", bufs=4) as ppre,
            tc.tile_pool(name="pmed", bufs=1) as pmed,
            tc.tile_pool(name="pw", bufs=1) as pw,
            tc.tile_pool(name="pg", bufs=1) as pg,
            tc.tile_pool(name="psc", bufs=1) as psc,
            tc.tile_pool(name="pp", bufs=1, space="PSUM") as pp,
        ):
            # ---- ND embeddings first: one contiguous load feeding the
            # first matmuls ----
            emb_all = psc.tile([128, 12, 128], F16, tag="emb", bufs=1)
            nc.sync.dma_start(
                out=emb_all[:].rearrange("p a c -> p (a c)"), in_=ndemb[:])

            # ---- constants ----
            bpf = psc.tile([128, 26], F32, tag="bpf", bufs=1)
            nc.sync.dma_start(out=bpf[:], in_=bias_pf[:])
            brow_g = psc.tile([1, 512], F16, tag="brow_g", bufs=1)
            nc.sync.dma_start(out=brow_g[:], in_=bias_rows[64:65, :])
            ones = psc.tile([1, 512], F16, tag="ones", bufs=1)
            nc.vector.memset(ones[:], 1.0)
            ident = psc.tile([128, 128], F16, tag="ident", bufs=1)
            make_identity(nc, ident[:])
            idxs = {}
            for nm, dd in (("1l", idx1l_d), ("1r", idx1r_d),
                           ("2a", idx2a_d), ("2b", idx2b_d)):
                t_ = psc.tile([128, dd.shape[1]], I16, tag=f"idx{nm}", bufs=1)
                nc.sync.dma_start(out=t_[:], in_=dd[:])
                idxs[nm] = t_

            # ---- ND path (128 entities) — issued first so AG-nd fires early ----
            nd_act = []
            for half in range(2):
                W_d = name_W if half == 0 else desc_W
                embs = [emb_all[:, 6 * half + ki, :] for ki in range(6)]
                for mo in range(6):
                    ps = pp.tile([128, 512], F32, tag="ps_mm", bufs=2, space="PSUM")
                    wstrip = pw.tile([128, 6, 128], F16, tag="wnd6", bufs=3)
                    nc.sync.dma_start(
                        out=wstrip[:].rearrange("p a m -> p (a m)"),
                        in_=W_d[:, 768 * mo:768 * (mo + 1)])
                    for ki in range(6):
                        nc.tensor.matmul(ps[:, :128], lhsT=wstrip[:, ki, :],
                                         rhs=embs[ki],
                                         start=(ki == 0), stop=(ki == 5))
                    a = psc.tile([128, 128], F16, tag="ndact", bufs=12,
                                 name=f"ndact{half}_{mo}")
                    nc.scalar.activation(a[:], ps[:, :128], ACTF.Prelu,
                                         bias=bpf[:, 6 * half + mo:6 * half + mo + 1],
                                         alpha=SLOPE)
                    nd_act.append(a)
            for mo in range(4):
                ps = pp.tile([128, 512], F32, tag="ps_mm", bufs=2, space="PSUM")
                wstrip = pw.tile([128, 12, 128], F16, tag="wnd12", bufs=2)
                nc.sync.dma_start(
                    out=wstrip[:].rearrange("p a m -> p (a m)"),
                    in_=fus_nd[:, 1536 * mo:1536 * (mo + 1)])
                for ki in range(12):
                    nc.tensor.matmul(ps[:, :128], lhsT=wstrip[:, ki, :],
                                     rhs=nd_act[ki][:],
                                     start=(ki == 0), stop=(ki == 11))
                r_ = psc.tile([128, 128], F16, tag="ndres", bufs=4, name=f"ndres{mo}")
                nc.vector.tensor_copy(out=r_[:], in_=ps[:, :128])
                nc.sync.dma_start(out=agnd_in[128 * mo:128 * (mo + 1), :], in_=r_[:])
            nc.gpsimd.collective_compute(
                "AllGather", ALU.bypass, replica_groups=RG,
                ins=[agnd_in[:]], outs=[agnd_out[:]])

            # ---- big activations (fp16), loaded behind the ND-path inputs ----
            xt = []
            for k in range(4):
                t = pbig.tile([128, R], F16, tag="bigA", bufs=8, name=f"xt{k}")
                nc.sync.dma_start(out=t[:], in_=x_t[128 * k:128 * (k + 1), :])
                xt.append(t)
            pre = []
            for k in range(4):
                t = ppre.tile([128, R], F16, tag="pre", bufs=4, name=f"pre{k}")
                nc.sync.dma_start(out=t[:], in_=pre_t_d[128 * k:128 * (k + 1), :])
                pre.append(t)

            # ---- weights for the dense pipeline ----
            womic = [pw.tile([128, 512], F16, tag="wres", bufs=20, name=f"womic{k}")
                     for k in range(4)]
            wfom = [pw.tile([128, 512], F16, tag="wres", bufs=20, name=f"wfom{k}")
                    for k in range(4)]
            wienc = [pw.tile([128, 512], F16, tag="wres", bufs=20, name=f"wienc{k}")
                     for k in range(4)]
            wenc = [pw.tile([128, 512], F16, tag="wres", bufs=20, name=f"wenc{k}")
                    for k in range(4)]
            wpe = [pw.tile([128, 512], F16, tag="wres", bufs=20, name=f"wpe{k}")
                   for k in range(4)]
            for k in range(4):
                nc.sync.dma_start(out=womic[k][:], in_=omic_W[128 * k:128 * (k + 1), :])
                nc.sync.dma_start(out=wfom[k][:], in_=fus_om[128 * k:128 * (k + 1), :])
                nc.sync.dma_start(out=wienc[k][:], in_=ienc_W[128 * k:128 * (k + 1), :])
                nc.sync.dma_start(out=wenc[k][:], in_=enc_W[128 * k:128 * (k + 1), :])
                nc.sync.dma_start(out=wpe[k][:], in_=wpe_W[128 * k:128 * (k + 1), :])

            # tiled cross_nd landed in SBUF once; injected into the fus chain
            nd_sb = [pmed.tile([128, 1024], F16, tag="ndsb", bufs=4, name=f"ndsb{k}")
                     for k in range(4)]
            for k in range(4):
                nc.sync.dma_start(
                    out=nd_sb[k][:].rearrange("p (r c) -> p r c", r=NCORE),
                    in_=agnd_out[:].rearrange("(r q p) c -> q p r c",
                                              r=NCORE, q=4)[k])

            # ---- omic + fus (+ nd inject) -> cross_c^T; m1 per j-group ----
            cross = [pbig.tile([128, R], F16, tag="bigA", bufs=8, name=f"cross{k}")
                     for k in range(4)]
            # ko row lands early; fus copies for k=3 skip partition 127
            nc.sync.dma_start(out=cross[3][127:128, :], in_=x_t[511:512, :])
            for j in range(4):
                sl = slice(512 * j, 512 * (j + 1))
                e0 = 512 * (j % 2)
                om_j = []
                for k in range(4):
                    ps = pp.tile([128, 512], F32, tag="ps_mm", bufs=2, space="PSUM")
                    for ki in range(4):
                        nc.tensor.matmul(ps[:], lhsT=womic[ki][:, 128 * k:128 * (k + 1)],
                                         rhs=xt[ki][:, sl], start=(ki == 0), stop=(ki == 3))
                    a = pmed.tile([128, 512], F16, tag="omj", bufs=8)
                    nc.scalar.activation(a[:], ps[:], ACTF.Prelu,
                                         bias=bpf[:, 12 + k:13 + k], alpha=SLOPE)
                    om_j.append(a)
                for k in range(4):
                    ps = pp.tile([128, 512], F32, tag="ps_mm", bufs=2, space="PSUM")
                    for ki in range(4):
                        nc.tensor.matmul(ps[:], lhsT=wfom[ki][:, 128 * k:128 * (k + 1)],
                                         rhs=om_j[ki][:], start=(ki == 0),
                                         stop=False)
                    # + tiled cross_nd (fus_b asserted zero)
                    nc.tensor.matmul(ps[:], lhsT=ident[:],
                                     rhs=nd_sb[k][:, e0:e0 + 512],
                                     start=False, stop=True)
                    np_ = 127 if k == 3 else 128
                    nc.scalar.activation(cross[k][0:np_, sl], ps[0:np_, :], ACTF.Copy)
                # m1 for this j-group's 4 row tiles
                for t in range(4 * j, 4 * j + 4):
                    tsl = slice(128 * t, 128 * (t + 1))
                    ps = pp.tile([128, 512], F32, tag="ps_seg", bufs=2, space="PSUM")
                    for ki in range(4):
                        nc.tensor.matmul(ps[:], lhsT=cross[ki][:, tsl], rhs=wienc[ki][:],
                                         start=(ki == 0), stop=(ki == 3))
                    h = pmed.tile([128, 512], F16, tag="m1h", bufs=3)
                    nc.vector.tensor_copy(out=h[:], in_=ps[:])
                    nc.sync.dma_start(out=ag1_in[tsl, :], in_=h[:])
                    h8 = pmed.tile([128, 512], F8, tag="m1h8", bufs=3)
                    nc.scalar.activation(h8[:], ps[:], ACTF.Copy)
                    nc.sync.dma_start(
                        out=ag8_in[32 * t:32 * (t + 1), :].rearrange(
                            "a (b f) -> (a b) f", b=4),
                        in_=h8[:])
            if DEBUG:
                for k in range(4):
                    nc.sync.dma_start(out=dbg_cross[128 * k:128 * (k + 1), :],
                                      in_=cross[k][:])
            nc.gpsimd.collective_compute(
                "AllGather", ALU.bypass, replica_groups=RG,
                ins=[ag8_in[:]], outs=[ag8_out[:]])

            # ---- m2 dense part during the AG window:
            #      m2d = x_c @ enc_W + pre_c @ (pre_W @ enc_W) ----
            m2d = [pmed.tile([128, 512], F16, tag="m2d", bufs=NT, name=f"m2d{t}")
                   for t in range(NT)]
            for t in range(NT):
                tsl = slice(128 * t, 128 * (t + 1))
                ps = pp.tile([128, 512], F32, tag="ps_mm", bufs=2, space="PSUM")
                for ki in range(4):
                    nc.tensor.matmul(ps[:], lhsT=xt[ki][:, tsl], rhs=wenc[ki][:],
                                     start=(ki == 0), stop=False)
                for ki in range(4):
                    nc.tensor.matmul(ps[:], lhsT=pre[ki][:, tsl], rhs=wpe[ki][:],
                                     start=False, stop=(ki == 3))
                nc.scalar.activation(m2d[t][:], ps[:], ACTF.Copy)

            # ---- generic gather+scatter ----
            def _bounds(nch_t):
                b = []
                for t_id, nch in enumerate(nch_t):
                    for j in range(nch):
                        b.append((t_id, j == 0, j == nch - 1))
                return b

            def scatter(src_dram, idx_t, sel_d, sel_dt, nchunks, tile_bounds,
                        psum_tag, gbufs_n, sfx="", on_a=None, on_b=None,
                        psum_bufs=2):
                """Two-phase pipelined per-tile post-processing: on_a(t, ps)
                fires one tile late (at the next tile's first chunk) and
                returns a context; on_b(ctx) fires another tile later. The
                delay keeps the tensor queue from stalling on cross-engine
                dependencies of the post-processing."""
                out_psums = []
                qa, qb = [], []
                ps = None
                src_ap = src_dram if isinstance(src_dram, bass.AP) else src_dram[:]

                def pump():
                    if qb and on_b is not None:
                        on_b(qb.pop(0))
                    if qa:
                        t_id_, ps_ = qa.pop(0)
                        if on_a is not None:
                            ctx = on_a(t_id_, ps_)
                            if on_b is not None:
                                qb.append(ctx)
                cur = 0
                i = 0
                for w, s in enumerate(_wave_sizes(nchunks)):
                    g = pg.tile([128, WAVE, 512], sel_dt, tag="gath" + sfx,
                                bufs=gbufs_n)
                    nc.gpsimd.dma_gather(
                        g[:, :s, :], src_ap, idx_t[:, 8 * cur:8 * (cur + s)],
                        s * 128, s * 128, 512,
                        single_packet=True, queue_num=w % 4)
                    sw = pg.tile([128, WAVE, 128], sel_dt, tag="selw" + sfx,
                                 bufs=gbufs_n)
                    nc.sync.dma_start(
                        out=sw[:, :s, :].rearrange("p a d -> p (a d)"),
                        in_=sel_d[:, 128 * cur:128 * (cur + s)])
                    for slot in range(s):
                        t_id, first, last = tile_bounds[i]
                        i += 1
                        if first:
                            pump()
                            ps = pp.tile([128, 512], F32, tag=psum_tag,
                                         bufs=psum_bufs, space="PSUM")
                        nc.tensor.matmul(ps[:], lhsT=sw[:, slot, :],
                                         rhs=g[:, slot, :],
                                         start=first, stop=last)
                        if last:
                            out_psums.append((t_id, ps))
                            qa.append((t_id, ps))
                    cur += s
                pump()
                pump()
                return out_psums

            # ---- gconv1 local-src edges: gathered from ag1_in DURING AG1 ----
            uacc = [pmed.tile([128, 512], F16, tag="uacc", bufs=NT, name=f"uacc{t}")
                    for t in range(NT)]
            seg1l = scatter(ag1_in, idxs["1l"], sel1l_d, F16, C1L,
                            _bounds(nch1l_t), "ps_seg", 2)
            for t_id, ps in seg1l:
                nc.vector.tensor_copy(out=uacc[t_id][:], in_=ps[:])

            # ---- gconv1 remote edges; per tile: u -> lrelu(u)^T strips ->
            #      m2 = m2d + lrelu(u) @ enc_W -> DRAM for gconv2 ----
            ag8_rows = ag8_out[:].rearrange("a (b f) -> (a b) f", b=4)

            def z_a(t_id, ps):
                useg = pmed.tile([128, 512], F16, tag="useg", bufs=3)
                nc.vector.tensor_tensor(out=useg[:], in0=ps[:],
                                        in1=uacc[t_id][:], op=ALU.add)
                pst = pp.tile([128, 512], F16, tag="ps_t", bufs=2, space="PSUM")
                for k in range(4):
                    nc.tensor.transpose(
                        out=pst[:, 128 * k:128 * (k + 1)],
                        in_=useg[:, 128 * k:128 * (k + 1)], identity=ident[:])
                ut = pmed.tile([128, 512], F16, tag="ut", bufs=3)
                for k in range(4):
                    nc.scalar.activation(ut[:, 128 * k:128 * (k + 1)],
                                         pst[:, 128 * k:128 * (k + 1)], ACTF.Prelu,
                                         bias=bpf[:, 16 + k:17 + k], alpha=SLOPE)
                return (t_id, ut)

            def z_b(ctx):
                t_id, ut = ctx
                ps2 = pp.tile([128, 512], F32, tag="ps_m2", bufs=2, space="PSUM")
                for ki in range(4):
                    nc.tensor.matmul(ps2[:], lhsT=ut[:, 128 * ki:128 * (ki + 1)],
                                     rhs=wenc[ki][:], start=(ki == 0), stop=(ki == 3))
                h = pmed.tile([128, 512], F16, tag="m2h", bufs=3)
                nc.vector.tensor_tensor(out=h[:], in0=ps2[:], in1=m2d[t_id][:],
                                        op=ALU.add)
                if DEBUG:
                    nc.sync.dma_start(
                        out=dbg_m2[128 * t_id:128 * (t_id + 1), :], in_=h[:])
                if t_id < 12:
                    nc.sync.dma_start(
                        out=m2_a[128 * t_id:128 * (t_id + 1), :], in_=h[:])
                else:
                    nc.sync.dma_start(
                        out=m2_b[128 * (t_id - 12):128 * (t_id - 11), :], in_=h[:])

            scatter(ag8_rows, idxs["1r"], sel1r_d, F8, C1R,
                    _bounds(nch1r_t), "ps_seg", 4, sfx="8", on_a=z_a, on_b=z_b)

            # ---- gconv2: source-side partials over 1024 slots, transposed to
            #      [feat, slot] per slot tile, then ReduceScatter ----
            acc2 = [pmed.tile([128, 512], F16, tag="acc2", bufs=8, name=f"acc2{t}")
                    for t in range(8)]
            seg2a = scatter(m2_a, idxs["2a"], sel2a_d, F16, C2A,
                            _bounds(nch2a_t), "ps_seg", 2)
            for t_id, ps in seg2a:
                nc.vector.tensor_copy(out=acc2[t_id][:], in_=ps[:])

            def rs_tile(t_id, ps):
                pc = pmed.tile([128, 512], F16, tag="m2h", bufs=3)
                nc.vector.tensor_tensor(out=pc[:], in0=ps[:], in1=acc2[t_id][:],
                                        op=ALU.add)
                pst = pp.tile([128, 512], F16, tag="ps_t", bufs=2, space="PSUM")
                for k in range(4):
                    nc.tensor.transpose(
                        out=pst[:, 128 * k:128 * (k + 1)],
                        in_=pc[:, 128 * k:128 * (k + 1)], identity=ident[:])
                zb = pmed.tile([128, 512], F16, tag="zb", bufs=2)
                nc.scalar.activation(zb[:], pst[:], ACTF.Copy)
                nc.sync.dma_start(
                    out=rs_in[512 * t_id:512 * (t_id + 1), :].rearrange(
                        "(a p) c -> p a c", a=4),
                    in_=zb[:].rearrange("p (a c) -> p a c", a=4))

            scatter(m2_b, idxs["2b"], sel2b_d, F16, C2B,
                    _bounds(nch2b_t), "ps_seg", 2, on_a=rs_tile)
            nc.gpsimd.collective_compute(
                "ReduceScatter", ALU.add, replica_groups=RG,
                ins=[rs_in[:]], outs=[rs_out[:]])

            # ---- zk^T straight off the wire + readout ----
            zk16 = pmed.tile([128, 512], F16, tag="zb", bufs=2)
            nc.sync.dma_start(
                out=zk16[:].rearrange("p (a c) -> p a c", a=4),
                in_=rs_out[:].rearrange("(a p) c -> p a c", a=4))
            zkt = pmed.tile([128, 512], F16, tag="zkt", bufs=1)
            for k in range(4):
                kl = slice(128 * k, 128 * (k + 1))
                nc.scalar.activation(zkt[:, kl], zk16[:, kl], ACTF.Prelu,
                                     bias=bpf[:, 20 + k:21 + k], alpha=SLOPE)
            if DEBUG:
                zkf = pmed.tile([128, 512], F32, tag="zkf", bufs=1)
                for k in range(4):
                    ps = pp.tile([128, 512], F16, tag="ps_t", bufs=2, space="PSUM")
                    nc.tensor.transpose(out=ps[:, :128],
                                        in_=zkt[:, 128 * k:128 * (k + 1)],
                                        identity=ident[:])
                    nc.vector.tensor_copy(out=zkf[:, 128 * k:128 * (k + 1)],
                                          in_=ps[:, :128])
                nc.sync.dma_start(out=dbg_zk[:], in_=zkf[:])

            wg1 = [pw.tile([128, 512], F16, tag="wres", bufs=20, name=f"wg1{k}")
                   for k in range(4)]
            for k in range(4):
                nc.sync.dma_start(out=wg1[k][:], in_=gate_W1[128 * k:128 * (k + 1), :])
            w2r = psc.tile([128, 8], F16, tag="w2r", bufs=1)
            nc.sync.dma_start(out=w2r[:], in_=gw2reg[:])
            s1t = pmed.tile([128, 512], F16, tag="s1t", bufs=1)
            for ko_ in range(4):
                ps = pp.tile([128, 512], F32, tag="ps_mm", bufs=2, space="PSUM")
                for ki in range(4):
                    nc.tensor.matmul(ps[:, :128],
                                     lhsT=wg1[ki][:, 128 * ko_:128 * (ko_ + 1)],
                                     rhs=zkt[:, 128 * ki:128 * (ki + 1)],
                                     start=(ki == 0), stop=False)
                nc.tensor.matmul(ps[:, :128],
                                 lhsT=brow_g[:, 128 * ko_:128 * (ko_ + 1)],
                                 rhs=ones[:, :128], start=False, stop=True)
                nc.scalar.activation(s1t[:, 128 * ko_:128 * (ko_ + 1)], ps[:, :128],
                                     ACTF.Tanh)
            ps_sc = pp.tile([128, 512], F32, tag="ps_mm", bufs=2, space="PSUM")
            for ki in range(4):
                nc.tensor.matmul(ps_sc[:1, :128], lhsT=w2r[:, 2 * ki:2 * ki + 1],
                                 rhs=s1t[:, 128 * ki:128 * (ki + 1)],
                                 start=(ki == 0), stop=(ki == 3))
            ps_tr = pp.tile([128, 512], F32, tag="ps_seg", bufs=2, space="PSUM")
            for ki in range(4):
                nc.tensor.matmul(ps_tr[:1, :128], lhsT=w2r[:, 2 * ki + 1:2 * ki + 2],
                                 rhs=zkt[:, 128 * ki:128 * (ki + 1)],
                                 start=(ki == 0), stop=(ki == 3))
            erow = psc.tile([1, 128], F32, tag="erow", bufs=1)
            nc.scalar.activation(erow[:], ps_sc[:1, :128], ACTF.Exp,
                                 bias=bpf[:1, 24:25])
            etrow = psc.tile([1, 128], F32, tag="etrow", bufs=1)
            nc.vector.tensor_tensor(out=etrow[:], in0=erow[:], in1=ps_tr[:1, :128],
                                    op=ALU.mult)
            sums = psc.tile([1, 4], F32, tag="sums", bufs=1)
            nc.vector.tensor_reduce(out=sums[:, 0:2],
                                    in_=etrow[:].rearrange("p (g x) -> p g x", g=2),
                                    axis=AX, op=ALU.add)
            nc.vector.tensor_reduce(out=sums[:, 2:4],
                                    in_=erow[:].rearrange("p (g x) -> p g x", g=2),
                                    axis=AX, op=ALU.add)
            res = psc.tile([1, 4], F32, tag="res", bufs=1)
            nc.vector.reciprocal(out=res[:, 2:4], in_=sums[:, 2:4])
            nc.vector.tensor_tensor(out=res[:, 0:2], in0=sums[:, 0:2],
                                    in1=res[:, 2:4], op=ALU.mult)
            nc.vector.tensor_scalar(out=res[:, 0:2], in0=res[:, 0:2],
                                    scalar1=bpf[:1, 25:26], scalar2=None, op0=ALU.add)
            nc.sync.dma_start(out=out_d[:], in_=res[:, 0:2])

    nc.compile()
    return nc


def _ensure_ntff_hook():
    """Inject antenv.axon_hooks (absent in this image) so trace=True works."""
    import sys, types
    try:
        from antenv.axon_hooks import get_axon_ntff_profile_hook  # noqa
        return
    except ImportError:
        pass
    import antenv
    mod = types.ModuleType("antenv.axon_hooks")
    _state = {"hook": None}
    mod.set_axon_ntff_profile_hook = lambda h: _state.__setitem__("hook", h)
    mod.get_axon_ntff_profile_hook = lambda: _state["hook"]
    sys.modules["antenv.axon_hooks"] = mod
    antenv.axon_hooks = mod
    from trn_agent_boot.trn_boot import _ntff_profile_via_ctypes
    mod.set_axon_ntff_profile_hook(
        _ntff_profile_via_ctypes("/opt/axon/libaxon_pjrt.so"))


# ---------------------------------------------------------------------------
# host wrapper
# ---------------------------------------------------------------------------

def kernel(**inputs):
    f32 = lambda k: np.asarray(inputs[k], np.float32)
    x = f32("x"); pre_x = f32("pre_x")
    edge_index = np.asarray(inputs["edge_index"], np.int64)
    internal_edge_index = np.asarray(inputs["internal_edge_index"], np.int64)
    name_emb = f32("name_embeddings"); desc_emb = f32("desc_embeddings")
    ko_mask = np.asarray(inputs["ko_mask"], np.int64)
    bkm = np.asarray(inputs["batch_ko_masks"], np.int64)
    name_W = f32("name_W"); name_b = f32("name_b")
    desc_W = f32("desc_W"); desc_b = f32("desc_b")
    omic_W = f32("omic_W"); omic_b = f32("omic_b")
    fus_W = f32("fus_W"); fus_b = f32("fus_b")
    pre_W = f32("pre_W"); pre_b = f32("pre_b")
    ienc_W = f32("ienc_W"); ienc_b = f32("ienc_b")
    enc_W = f32("enc_W"); enc_b = f32("enc_b")
    gate_W1 = f32("gate_W1"); gate_b1 = f32("gate_b1")
    gate_W2 = f32("gate_W2"); gate_b2 = f32("gate_b2")
    reg_W = f32("reg_W"); reg_b = f32("reg_b")

    assert not fus_b.any() and not pre_b.any(), \
        "nonzero fus_b/pre_b not supported by this build"

    ko_feat = np.zeros(N, np.float32)
    ko_feat[ko_mask] = 1.0

    # ---- gconv2: source-sharded edges into the 1024 global KO slots ----
    slot_row = (bkm + np.arange(B)[:, None] * NE).reshape(-1)   # [1024]
    row2slots = {}
    for s_, r_ in enumerate(slot_row):
        row2slots.setdefault(int(r_), []).append(s_)
    def _pad_last(nch_t):
        nch_t[-1] += (-int(nch_t.sum())) % WAVE
        return tuple(int(v) for v in nch_t)

    s2_all, d2_all = edge_index[0], edge_index[1]
    m2mask = np.isin(d2_all, slot_row)
    per_core_2a = []   # sources in local rows [0, R/2)
    per_core_2b = []   # sources in local rows [R/2, R)
    needed = []        # per-core local rows whose z/m2 is actually consumed
    nch2a_t = np.ones(8, np.int64)
    nch2b_t = np.ones(8, np.int64)
    for c in range(NCORE):
        lo, hi = R * c, R * (c + 1)
        ss, ds = [], []
        for r_, sl_ in row2slots.items():
            if lo <= r_ < hi:
                for s_ in sl_:
                    ss.append(r_ - lo); ds.append(s_)
        mm = m2mask & (s2_all >= lo) & (s2_all < hi)
        for u, v in zip(s2_all[mm], d2_all[mm]):
            for s_ in row2slots[int(v)]:
                ss.append(int(u) - lo); ds.append(s_)
        src = np.array(ss, np.int64); dstl = np.array(ds, np.int64)
        nd = np.zeros(R, bool)
        nd[src] = True
        needed.append(nd)
        ha = src < 12 * 128
        per_core_2a.append((src[ha], dstl[ha]))
        per_core_2b.append((src[~ha] - 12 * 128, dstl[~ha]))
        nch2a_t = np.maximum(nch2a_t, -(-np.bincount(dstl[ha] >> 7, minlength=8) // 128))
        nch2b_t = np.maximum(nch2b_t, -(-np.bincount(dstl[~ha] >> 7, minlength=8) // 128))
    nch2a_t = _pad_last(nch2a_t)
    nch2b_t = _pad_last(nch2b_t)

    # ---- gconv1 edges (dst-sharded; self term added from local m1h).
    # Edges whose dst row never feeds gconv2 (not a slot row, not a source of
    # a slot edge) are dropped: their z rows are never read. Edges with a
    # LOCAL source are gathered from ag1_in during AG1. ----
    s1_all, d1_all = internal_edge_index[0], internal_edge_index[1]
    per_core_1l = []
    per_core_1r = []
    nch1l_t = np.ones(NT, np.int64)
    nch1r_t = np.ones(NT, np.int64)
    for c in range(NCORE):
        lo, hi = R * c, R * (c + 1)
        m = (d1_all >= lo) & (d1_all < hi)
        s1 = s1_all[m]; d1l = d1_all[m] - lo
        keep = needed[c][d1l]
        s1 = s1[keep]; d1l = d1l[keep]
        isloc = (s1 >= lo) & (s1 < hi)
        # the gconv self term rides the local pass as synthetic (r -> r) edges
        selfr = np.nonzero(needed[c])[0]
        ls = np.concatenate([s1[isloc] - lo, selfr])
        ld = np.concatenate([d1l[isloc], selfr])
        per_core_1l.append((ls, ld))
        per_core_1r.append((s1[~isloc], d1l[~isloc]))
        nch1l_t = np.maximum(
            nch1l_t, -(-np.bincount(ld >> 7, minlength=NT) // 128))
        nch1r_t = np.maximum(
            nch1r_t, -(-np.bincount(d1l[~isloc] >> 7, minlength=NT) // 128))
    nch1l_t = _pad_last(nch1l_t)
    nch1r_t = _pad_last(nch1r_t)

    nc = _build(nch1l_t, nch1r_t, nch2a_t, nch2b_t)

    import ml_dtypes
    f16 = np.float16
    f8 = ml_dtypes.float8_e4m3
    omic_Wp = _pad_w(omic_W, 512, 512)
    fus_ndp = _pad_w(fus_W[:2 * TX], 2 * TX, 512)
    fus_omp = _pad_w(fus_W[2 * TX:], 512, 512)
    wpe = pre_W @ enc_W                       # fold z-pre path into m2
    # [p, mo, ki, m] = W[ki*128+p, mo*128+m] so wstrip loads are contiguous
    name_Wr = np.ascontiguousarray(
        name_W.reshape(6, 128, 6, 128).transpose(1, 2, 0, 3).reshape(128, 6 * TX))
    desc_Wr = np.ascontiguousarray(
        desc_W.reshape(6, 128, 6, 128).transpose(1, 2, 0, 3).reshape(128, 6 * TX))
    fus_ndr = np.ascontiguousarray(
        fus_ndp.reshape(12, 128, 4, 128).transpose(1, 2, 0, 3).reshape(128, 6144))
    bias_pf = np.zeros((128, 26), np.float32)
    bias_pf[:, 0:6] = name_b.reshape(6, 128).T
    bias_pf[:, 6:12] = desc_b.reshape(6, 128).T
    bias_pf[:, 12:16] = _pad_w(omic_b[:, None], 512, 1).reshape(4, 128).T
    bias_pf[:, 16:20] = ienc_b.reshape(4, 128).T
    bias_pf[:, 20:24] = enc_b.reshape(4, 128).T
    bias_pf[:, 24] = float(gate_b2.reshape(-1)[0])
    bias_pf[:, 25] = float(reg_b.reshape(-1)[0])
    bias_rows = np.zeros((96, 512), np.float32)
    bias_rows[64, :] = gate_b1
    gw2 = np.concatenate([gate_W2, reg_W], axis=1).astype(np.float32)
    gw2 = np.ascontiguousarray(
        gw2.reshape(4, 128, 2).transpose(1, 0, 2).reshape(128, 8))

    shared = dict(
        name_W=name_Wr.astype(f16), desc_W=desc_Wr.astype(f16),
        omic_W=omic_Wp.astype(f16), fus_nd=fus_ndr.astype(f16),
        fus_om=fus_omp.astype(f16), ienc_W=ienc_W.astype(f16),
        wpe_W=wpe.astype(f16), enc_W=enc_W.astype(f16),
        gate_W1=gate_W1.astype(f16), gw2reg=gw2.astype(f16), bias_pf=bias_pf,
        bias_rows=bias_rows.astype(f16),
    )

    in_maps = []
    for c in range(NCORE):
        lo, hi = R * c, R * (c + 1)
        x_t = np.concatenate([x[lo:hi].T, ko_feat[None, lo:hi]], 0)
        pre_t = np.concatenate([pre_x[lo:hi].T, ko_feat[None, lo:hi]], 0)
        ndemb = np.concatenate(
            [name_emb[128 * c:128 * (c + 1)].T, desc_emb[128 * c:128 * (c + 1)].T], 0)
        ndemb = ndemb.reshape(12, 128, 128).transpose(1, 0, 2).reshape(128, 12 * 128)
        i1l, dv1l = _chunk_edges_per_tile(*per_core_1l[c], nch1l_t)
        i1r, dv1r = _chunk_edges_per_tile(*per_core_1r[c], nch1r_t)
        i2a, dv2a = _chunk_edges_per_tile(*per_core_2a[c], nch2a_t)
        i2b, dv2b = _chunk_edges_per_tile(*per_core_2b[c], nch2b_t)
        in_maps.append(dict(
            x_t=np.ascontiguousarray(x_t).astype(f16),
            pre_t=np.ascontiguousarray(pre_t).astype(f16),
            ndemb=np.ascontiguousarray(ndemb).astype(f16),
            idx1l=_wrap_idx_waves(i1l), sel1l=_sel_from_dstv(dv1l, f16),
            idx1r=_wrap_idx_waves(i1r), sel1r=_sel_from_dstv(dv1r, f8),
            idx2a=_wrap_idx_waves(i2a), sel2a=_sel_from_dstv(dv2a, f16),
            idx2b=_wrap_idx_waves(i2b), sel2b=_sel_from_dstv(dv2b, f16),
            **shared,
        ))

    if TRACE:
        _ensure_ntff_hook()
    res = run_bass_kernel_spmd(nc, in_maps, core_ids=list(range(NCORE)),
                               trace=TRACE, **(TRACE_KW or {}))
    kernel._last = res
    out = np.zeros(B, np.float32)
    for c in range(NCORE):
        out[2 * c:2 * c + 2] = res.results[c]["out"][0]
    return out


# revision 18
# speedup vs baseline: 1.0609x; 1.0609x over previous
"""Trainium2 Bass kernel for nn_MOTASG_KO_Reg (ragged graph-conv KO regression).

Strategy (8 NeuronCores, data-parallel over node rows):
  - N=16384 nodes = 16 batch samples x 1024 entities. Core c owns rows
    [2048c, 2048c+2048) = batch samples 2c, 2c+1.
  - Activations kept feature-major ("transposed", [feat, rows]) on chip so
    every linear is a native PE matmul (fp16 operands, fp32 PSUM).
  - name/desc path computed once on 128 entities/core, AllGathered, folded
    into cross INSIDE the fus PSUM chain via identity-inject matmuls.
  - z is never materialized: m2 = z @ enc_W is expanded as
      m2 = x_c @ enc_W + pre_c @ (pre_W @ enc_W) + lrelu(u) @ enc_W
    with pre_W @ enc_W precomputed on host. The dense part (m2d) runs
    during the fp8 AllGather window; the u part joins per tile after the
    gconv1 scatter.
  - gconv1 segment-sum via dma_gather + one-hot scatter matmuls in PSUM.
    Edges whose dst never feeds gconv2 are pruned (exact). Local-source
    edges (plus the self term as synthetic r->r edges) gather from fp16
    ag1_in DURING the AllGather; remote edges gather fp8 rows from the
    fp8 AllGather output.
  - All leaky-relus run on the scalar engine (ACTF.Prelu, PSUM input,
    per-partition bias AP, alpha=slope) - one op instead of three
    vector ops. Transposes run in fp16 (4x fp32 PE rate).
  - gconv2 source-side partials into the 1024 KO slots; partials are
    PE-transposed to [feat, slot] BEFORE the fp16 ReduceScatter so the
    readout (gate + softmax + weighted sum + regression) starts directly
    from rs_out with no transpose chain.
"""

import functools
import numpy as np

import concourse.bacc as bacc
import concourse.mybir as mybir
import concourse.tile as tile
from concourse import bass
from concourse.bass_utils import run_bass_kernel_spmd
from concourse.masks import make_identity

NE, B, KO = 1024, 16, 64
TX, OM, D = 768, 511, 512
N = NE * B
NCORE = 8
R = N // NCORE        # 2048 rows per core
NT = R // 128         # 16 row tiles per core
SLOPE = 0.3
F32 = mybir.dt.float32
F16 = mybir.dt.float16
F8 = mybir.dt.float8e4
I16 = mybir.dt.int16
AX = mybir.AxisListType.X
ALU = mybir.AluOpType
ACTF = mybir.ActivationFunctionType

WAVE = 8  # max gather chunks per dma_gather call
WCOLS = WAVE * 8


def _wave_sizes(C):
    """Two 4-chunk lead waves cut first-data latency; 8-chunk steady state."""
    return [4, 4] + [8] * ((C - 8) // 8)
DEBUG = False
TRACE = False
TRACE_KW = None


# ---------------------------------------------------------------------------
# host-side edge preparation
# ---------------------------------------------------------------------------

def _chunk_edges_per_tile(src, dstl, nch_t):
    """Sort (src->dst_local) into per-destination-tile 128-edge chunks."""
    C = sum(nch_t)
    idx = np.zeros((C, 128), np.int16)
    dstv = np.full((C, 128), -2.0, np.float32)
    t_of = dstl >> 7
    base = 0
    for t, nch in enumerate(nch_t):
        m = t_of == t
        s = src[m]
        d = (dstl[m] - (t << 7)).astype(np.float32)
        n = len(s)
        assert n <= nch * 128, (n, nch)
        full, rem = divmod(n, 128)
        for j in range(full):
            idx[base + j] = s[j * 128:(j + 1) * 128]
            dstv[base + j] = d[j * 128:(j + 1) * 128]
        if rem:
            idx[base + full, :rem] = s[full * 128:]
            dstv[base + full, :rem] = d[full * 128:]
        base += nch
    return idx, dstv


def _wrap_idx_waves(idx_chunks):
    """[C,128] int16 -> [128, C*8] wrapped per dma_gather call."""
    C = idx_chunks.shape[0]
    cols = []
    cur = 0
    for s in _wave_sizes(C):
        lin = idx_chunks[cur:cur + s].reshape(-1)
        cur += s
        cols.append(np.tile(lin.reshape(-1, 16).T, (8, 1)))
    return np.ascontiguousarray(np.concatenate(cols, axis=1))


def _sel_from_dstv(dstv, dt):
    C = dstv.shape[0]
    sel = (dstv[:, :, None] == np.arange(128, dtype=np.float32)[None, None, :])
    sel = sel.astype(dt)          # [C, 128 slot, 128 dst]
    return np.ascontiguousarray(sel.transpose(1, 0, 2).reshape(128, C * 128))


def _pad_w(w, rows, cols):
    out = np.zeros((rows, cols), np.float32)
    out[:w.shape[0], :w.shape[1]] = w
    return out


# ---------------------------------------------------------------------------
# program builder
# ---------------------------------------------------------------------------

@functools.lru_cache(maxsize=4)
def _build(nch1l_t, nch1r_t, nch2a_t, nch2b_t):
    """gconv1 chunks per dst tile split into local-src (gathered from ag1_in
    during AG1) and remote-src sets; gconv2 chunks per slot tile split by
    m2-row half so its gathers overlap m2 production. Totals are multiples
    of WAVE."""
    C1L = sum(nch1l_t)
    C1R = sum(nch1r_t)
    C2A = sum(nch2a_t)
    C2B = sum(nch2b_t)
    nc = bacc.Bacc("TRN2", num_swdge_queues=4)

    def din(name, shape, dtype=F16):
        return nc.dram_tensor(name, shape, dtype, kind="ExternalInput")

    x_t = din("x_t", [512, R])                  # [x | ko]^T fp16
    pre_t_d = din("pre_t", [512, R])
    ndemb = din("ndemb", [128, 12 * 128])
    # pre-transposed on host: [p, mo, ki, m] = W[ki*128+p, mo*128+m]
    name_W = din("name_W", [128, 6 * TX])
    desc_W = din("desc_W", [128, 6 * TX])
    omic_W = din("omic_W", [512, 512])
    fus_nd = din("fus_nd", [128, 4 * 12 * 128])
    fus_om = din("fus_om", [512, 512])
    ienc_W = din("ienc_W", [512, 512])
    wpe_W = din("wpe_W", [512, 512])            # pre_W @ enc_W (host)
    enc_W = din("enc_W", [512, 512])
    gate_W1 = din("gate_W1", [512, 512], F16)
    gw2reg = din("gw2reg", [128, 8], F16)
    bias_pf = din("bias_pf", [128, 26], F32)
    bias_rows = din("bias_rows", [96, 512], F16)
    idx1l_d = din("idx1l", [128, (C1L // WAVE) * WCOLS], I16)
    sel1l_d = din("sel1l", [128, C1L * 128], F16)
    idx1r_d = din("idx1r", [128, (C1R // WAVE) * WCOLS], I16)
    sel1r_d = din("sel1r", [128, C1R * 128], F8)
    idx2a_d = din("idx2a", [128, (C2A // WAVE) * WCOLS], I16)
    sel2a_d = din("sel2a", [128, C2A * 128], F16)
    idx2b_d = din("idx2b", [128, (C2B // WAVE) * WCOLS], I16)
    sel2b_d = din("sel2b", [128, C2B * 128], F16)
    out_d = nc.dram_tensor("out", [1, 2], F32, kind="ExternalOutput")

    agnd_in = nc.dram_tensor("agnd_in", [512, 128], F16)
    agnd_out = nc.dram_tensor("agnd_out", [NCORE * 512, 128], F16, addr_space="Shared")
    ag1_in = nc.dram_tensor("ag1_in", [R, 512], F16)
    ag8_in = nc.dram_tensor("ag8_in", [R // 4, 4 * 512], F8)
    ag8_out = nc.dram_tensor("ag8_out", [N // 4, 4 * 512], F8, addr_space="Shared")
    m2_a = nc.dram_tensor("m2_a", [12 * 128, 512], F16)
    m2_b = nc.dram_tensor("m2_b", [4 * 128, 512], F16)
    rs_in = nc.dram_tensor("rs_in", [NCORE * 32, 4 * 512], F16)
    rs_out = nc.dram_tensor("rs_out", [32, 4 * 512], F16)
    RG = [list(range(NCORE))]

    if DEBUG:
        dbg_cross = nc.dram_tensor("dbg_cross", [512, R], F16, kind="ExternalOutput")
        dbg_m2 = nc.dram_tensor("dbg_m2", [R, 512], F16, kind="ExternalOutput")
        dbg_zk = nc.dram_tensor("dbg_zk", [128, 512], F32, kind="ExternalOutput")

    with tile.TileContext(nc) as tc:
        with (
            tc.tile_pool(name="pbig", bufs=8) as pbig,
            tc.tile_pool(name="ppre", bufs=4) as ppre,
            tc.tile_pool(name="pmed", bufs=1) as pmed,
            tc.tile_pool(name="pw", bufs=1) as pw,
            tc.tile_pool(name="pg", bufs=1) as pg,
            tc.tile_pool(name="psc", bufs=1) as psc,
            tc.tile_pool(name="pp", bufs=1, space="PSUM") as pp,
        ):
            # ---- ND embeddings first: one contiguous load feeding the
            # first matmuls ----
            emb_all = psc.tile([128, 12, 128], F16, tag="emb", bufs=1)
            nc.sync.dma_start(
                out=emb_all[:].rearrange("p a c -> p (a c)"), in_=ndemb[:])

            # ---- constants ----
            bpf = psc.tile([128, 26], F32, tag="bpf", bufs=1)
            nc.sync.dma_start(out=bpf[:], in_=bias_pf[:])
            brow_g = psc.tile([1, 512], F16, tag="brow_g", bufs=1)
            nc.sync.dma_start(out=brow_g[:], in_=bias_rows[64:65, :])
            ones = psc.tile([1, 512], F16, tag="ones", bufs=1)
            nc.vector.memset(ones[:], 1.0)
            ident = psc.tile([128, 128], F16, tag="ident", bufs=1)
            make_identity(nc, ident[:])
            idxs = {}
            for nm, dd in (("1l", idx1l_d), ("1r", idx1r_d),
                           ("2a", idx2a_d), ("2b", idx2b_d)):
                t_ = psc.tile([128, dd.shape[1]], I16, tag=f"idx{nm}", bufs=1)
                nc.sync.dma_start(out=t_[:], in_=dd[:])
                idxs[nm] = t_

            # ---- ND path (128 entities) — issued first so AG-nd fires early ----
            nd_act = []
            for half in range(2):
                W_d = name_W if half == 0 else desc_W
                embs = [emb_all[:, 6 * half + ki, :] for ki in range(6)]
                for mo in range(6):
                    ps = pp.tile([128, 512], F32, tag="ps_mm", bufs=2, space="PSUM")
                    wstrip = pw.tile([128, 6, 128], F16, tag="wnd6", bufs=2)
                    nc.sync.dma_start(
                        out=wstrip[:].rearrange("p a m -> p (a m)"),
                        in_=W_d[:, 768 * mo:768 * (mo + 1)])
                    for ki in range(6):
                        nc.tensor.matmul(ps[:, :128], lhsT=wstrip[:, ki, :],
                                         rhs=embs[ki],
                                         start=(ki == 0), stop=(ki == 5))
                    a = psc.tile([128, 128], F16, tag="ndact", bufs=12,
                                 name=f"ndact{half}_{mo}")
                    nc.scalar.activation(a[:], ps[:, :128], ACTF.Prelu,
                                         bias=bpf[:, 6 * half + mo:6 * half + mo + 1],
                                         alpha=SLOPE)
                    nd_act.append(a)
            for mo in range(4):
                ps = pp.tile([128, 512], F32, tag="ps_mm", bufs=2, space="PSUM")
                wstrip = pw.tile([128, 12, 128], F16, tag="wnd12", bufs=2)
                nc.sync.dma_start(
                    out=wstrip[:].rearrange("p a m -> p (a m)"),
                    in_=fus_nd[:, 1536 * mo:1536 * (mo + 1)])
                for ki in range(12):
                    nc.tensor.matmul(ps[:, :128], lhsT=wstrip[:, ki, :],
                                     rhs=nd_act[ki][:],
                                     start=(ki == 0), stop=(ki == 11))
                r_ = psc.tile([128, 128], F16, tag="ndres", bufs=4, name=f"ndres{mo}")
                nc.vector.tensor_copy(out=r_[:], in_=ps[:, :128])
                nc.sync.dma_start(out=agnd_in[128 * mo:128 * (mo + 1), :], in_=r_[:])
            nc.gpsimd.collective_compute(
                "AllGather", ALU.bypass, replica_groups=RG,
                ins=[agnd_in[:]], outs=[agnd_out[:]])

            # ---- weights first (small; unblock omic/fus/m1/m2d early) ----
            womic = [pw.tile([128, 512], F16, tag="wres", bufs=20, name=f"womic{k}")
                     for k in range(4)]
            wfom = [pw.tile([128, 512], F16, tag="wres", bufs=20, name=f"wfom{k}")
                    for k in range(4)]
            wienc = [pw.tile([128, 512], F16, tag="wres", bufs=20, name=f"wienc{k}")
                     for k in range(4)]
            wenc = [pw.tile([128, 512], F16, tag="wres", bufs=20, name=f"wenc{k}")
                    for k in range(4)]
            wpe = [pw.tile([128, 512], F16, tag="wres", bufs=20, name=f"wpe{k}")
                   for k in range(4)]
            for k in range(4):
                nc.sync.dma_start(out=womic[k][:], in_=omic_W[128 * k:128 * (k + 1), :])
                nc.sync.dma_start(out=wfom[k][:], in_=fus_om[128 * k:128 * (k + 1), :])
                nc.sync.dma_start(out=wienc[k][:], in_=ienc_W[128 * k:128 * (k + 1), :])
                nc.sync.dma_start(out=wenc[k][:], in_=enc_W[128 * k:128 * (k + 1), :])
                nc.sync.dma_start(out=wpe[k][:], in_=wpe_W[128 * k:128 * (k + 1), :])

            # ---- big activations (fp16) ----
            xt = []
            for k in range(4):
                t = pbig.tile([128, R], F16, tag="bigA", bufs=8, name=f"xt{k}")
                nc.sync.dma_start(out=t[:], in_=x_t[128 * k:128 * (k + 1), :])
                xt.append(t)
            pre = []
            for k in range(4):
                t = ppre.tile([128, R], F16, tag="pre", bufs=4, name=f"pre{k}")
                nc.sync.dma_start(out=t[:], in_=pre_t_d[128 * k:128 * (k + 1), :])
                pre.append(t)

            # tiled cross_nd landed in SBUF once; injected into the fus chain.
            # Loaded via the gpsimd (SWDGE) queue: it waits on AG-nd, and on
            # the sync queue that wait would head-of-line block the pre loads.
            nd_sb = [pmed.tile([128, 1024], F16, tag="ndsb", bufs=4, name=f"ndsb{k}")
                     for k in range(4)]
            for k in range(4):
                nc.gpsimd.dma_start(
                    out=nd_sb[k][:].rearrange("p (r c) -> p r c", r=NCORE),
                    in_=agnd_out[:].rearrange("(r q p) c -> q p r c",
                                              r=NCORE, q=4)[k])

            # ---- omic + fus (+ nd inject) -> cross_c^T; m1 per j-group ----
            cross = [pbig.tile([128, R], F16, tag="bigA", bufs=8, name=f"cross{k}")
                     for k in range(4)]
            # ko row lands early; fus copies for k=3 skip partition 127
            nc.sync.dma_start(out=cross[3][127:128, :], in_=x_t[511:512, :])
            for j in range(4):
                sl = slice(512 * j, 512 * (j + 1))
                e0 = 512 * (j % 2)
                om_j = []
                for k in range(4):
                    ps = pp.tile([128, 512], F32, tag="ps_mm", bufs=2, space="PSUM")
                    for ki in range(4):
                        nc.tensor.matmul(ps[:], lhsT=womic[ki][:, 128 * k:128 * (k + 1)],
                                         rhs=xt[ki][:, sl], start=(ki == 0), stop=(ki == 3))
                    a = pmed.tile([128, 512], F16, tag="omj", bufs=8)
                    nc.scalar.activation(a[:], ps[:], ACTF.Prelu,
                                         bias=bpf[:, 12 + k:13 + k], alpha=SLOPE)
                    om_j.append(a)
                for k in range(4):
                    ps = pp.tile([128, 512], F32, tag="ps_mm", bufs=2, space="PSUM")
                    for ki in range(4):
                        nc.tensor.matmul(ps[:], lhsT=wfom[ki][:, 128 * k:128 * (k + 1)],
                                         rhs=om_j[ki][:], start=(ki == 0),
                                         stop=False)
                    # + tiled cross_nd (fus_b asserted zero)
                    nc.tensor.matmul(ps[:], lhsT=ident[:],
                                     rhs=nd_sb[k][:, e0:e0 + 512],
                                     start=False, stop=True)
                    np_ = 127 if k == 3 else 128
                    nc.scalar.activation(cross[k][0:np_, sl], ps[0:np_, :], ACTF.Copy)
                # m1 for this j-group's 4 row tiles
                for t in range(4 * j, 4 * j + 4):
                    tsl = slice(128 * t, 128 * (t + 1))
                    ps = pp.tile([128, 512], F32, tag="ps_seg", bufs=2, space="PSUM")
                    for ki in range(4):
                        nc.tensor.matmul(ps[:], lhsT=cross[ki][:, tsl], rhs=wienc[ki][:],
                                         start=(ki == 0), stop=(ki == 3))
                    h = pmed.tile([128, 512], F16, tag="m1h", bufs=2)
                    nc.vector.tensor_copy(out=h[:], in_=ps[:])
                    nc.sync.dma_start(out=ag1_in[tsl, :], in_=h[:])
                    h8 = pmed.tile([128, 512], F8, tag="m1h8", bufs=2)
                    nc.scalar.activation(h8[:], ps[:], ACTF.Copy)
                    nc.sync.dma_start(
                        out=ag8_in[32 * t:32 * (t + 1), :].rearrange(
                            "a (b f) -> (a b) f", b=4),
                        in_=h8[:])
            if DEBUG:
                for k in range(4):
                    nc.sync.dma_start(out=dbg_cross[128 * k:128 * (k + 1), :],
                                      in_=cross[k][:])
            nc.gpsimd.collective_compute(
                "AllGather", ALU.bypass, replica_groups=RG,
                ins=[ag8_in[:]], outs=[ag8_out[:]])

            # ---- generic gather+scatter ----
            def _bounds(nch_t):
                b = []
                for t_id, nch in enumerate(nch_t):
                    for j in range(nch):
                        b.append((t_id, j == 0, j == nch - 1))
                return b

            def scatter(src_dram, idx_t, sel_d, sel_dt, nchunks, tile_bounds,
                        psum_tag, gbufs_n, sfx="", on_a=None, on_b=None,
                        preload_sel=False):
                """Two-phase pipelined per-tile post-processing: on_a(t, ps)
                fires one tile late (at the next tile's first chunk) and
                returns a context; on_b(ctx) fires another tile later. The
                delay keeps the tensor queue from stalling on cross-engine
                dependencies of the post-processing. preload_sel issues every
                sel-wave DMA up front so later sel loads are never head-of-line
                blocked behind result DMAs on the sync queue."""
                out_psums = []
                qa, qb = [], []
                ps = None
                src_ap = src_dram if isinstance(src_dram, bass.AP) else src_dram[:]
                wsz = _wave_sizes(nchunks)

                sels = []
                if preload_sel:
                    cur = 0
                    for w, s in enumerate(wsz):
                        sw = pg.tile([128, WAVE, 128], sel_dt, tag="selw" + sfx,
                                     bufs=len(wsz))
                        nc.sync.dma_start(
                            out=sw[:, :s, :].rearrange("p a d -> p (a d)"),
                            in_=sel_d[:, 128 * cur:128 * (cur + s)])
                        sels.append(sw)
                        cur += s

                def pump():
                    if qb and on_b is not None:
                        on_b(qb.pop(0))
                    if qa:
                        t_id_, ps_ = qa.pop(0)
                        if on_a is not None:
                            ctx = on_a(t_id_, ps_)
                            if on_b is not None:
                                qb.append(ctx)
                cur = 0
                i = 0
                for w, s in enumerate(wsz):
                    g = pg.tile([128, WAVE, 512], sel_dt, tag="gath" + sfx,
                                bufs=gbufs_n)
                    nc.gpsimd.dma_gather(
                        g[:, :s, :], src_ap, idx_t[:, 8 * cur:8 * (cur + s)],
                        s * 128, s * 128, 512,
                        single_packet=True, queue_num=w % 4)
                    if preload_sel:
                        sw = sels[w]
                    else:
                        sw = pg.tile([128, WAVE, 128], sel_dt, tag="selw" + sfx,
                                     bufs=gbufs_n)
                        nc.sync.dma_start(
                            out=sw[:, :s, :].rearrange("p a d -> p (a d)"),
                            in_=sel_d[:, 128 * cur:128 * (cur + s)])
                    for slot in range(s):
                        t_id, first, last = tile_bounds[i]
                        i += 1
                        if first:
                            pump()
                            ps = pp.tile([128, 512], F32, tag=psum_tag,
                                         bufs=2, space="PSUM")
                        nc.tensor.matmul(ps[:], lhsT=sw[:, slot, :],
                                         rhs=g[:, slot, :],
                                         start=first, stop=last)
                        if last:
                            out_psums.append((t_id, ps))
                            qa.append((t_id, ps))
                    cur += s
                pump()
                pump()
                return out_psums

            # ---- gconv1 local-src edges: gathered from ag1_in DURING AG1.
            # Emitted BEFORE m2d so the tensor queue consumes the 1l waves as
            # they land (g double-buffer recycles; gathers never stall). ----
            uacc = [pmed.tile([128, 512], F16, tag="uacc", bufs=NT, name=f"uacc{t}")
                    for t in range(NT)]
            seg1l = scatter(ag1_in, idxs["1l"], sel1l_d, F16, C1L,
                            _bounds(nch1l_t), "ps_seg", 2)
            for t_id, ps in seg1l:
                nc.vector.tensor_copy(out=uacc[t_id][:], in_=ps[:])

            # ---- m2 dense part fills the rest of the AG window:
            #      m2d = x_c @ enc_W + pre_c @ (pre_W @ enc_W) ----
            m2d = [pmed.tile([128, 512], F16, tag="m2d", bufs=NT, name=f"m2d{t}")
                   for t in range(NT)]
            for t in range(NT):
                tsl = slice(128 * t, 128 * (t + 1))
                ps = pp.tile([128, 512], F32, tag="ps_mm", bufs=2, space="PSUM")
                for ki in range(4):
                    nc.tensor.matmul(ps[:], lhsT=xt[ki][:, tsl], rhs=wenc[ki][:],
                                     start=(ki == 0), stop=False)
                for ki in range(4):
                    nc.tensor.matmul(ps[:], lhsT=pre[ki][:, tsl], rhs=wpe[ki][:],
                                     start=False, stop=(ki == 3))
                nc.scalar.activation(m2d[t][:], ps[:], ACTF.Copy)

            # ---- gconv1 remote edges; per tile: u -> lrelu(u)^T strips ->
            #      m2 = m2d + lrelu(u) @ enc_W -> DRAM for gconv2 ----
            ag8_rows = ag8_out[:].rearrange("a (b f) -> (a b) f", b=4)

            def z_a(t_id, ps):
                useg = pmed.tile([128, 512], F16, tag="useg", bufs=2)
                nc.vector.tensor_tensor(out=useg[:], in0=ps[:],
                                        in1=uacc[t_id][:], op=ALU.add)
                pst = pp.tile([128, 512], F16, tag="ps_t", bufs=2, space="PSUM")
                for k in range(4):
                    nc.tensor.transpose(
                        out=pst[:, 128 * k:128 * (k + 1)],
                        in_=useg[:, 128 * k:128 * (k + 1)], identity=ident[:])
                ut = pmed.tile([128, 512], F16, tag="ut", bufs=2)
                for k in range(4):
                    nc.scalar.activation(ut[:, 128 * k:128 * (k + 1)],
                                         pst[:, 128 * k:128 * (k + 1)], ACTF.Prelu,
                                         bias=bpf[:, 16 + k:17 + k], alpha=SLOPE)
                return (t_id, ut)

            def z_b(ctx):
                t_id, ut = ctx
                ps2 = pp.tile([128, 512], F32, tag="ps_m2", bufs=2, space="PSUM")
                for ki in range(4):
                    nc.tensor.matmul(ps2[:], lhsT=ut[:, 128 * ki:128 * (ki + 1)],
                                     rhs=wenc[ki][:], start=(ki == 0), stop=(ki == 3))
                h = pmed.tile([128, 512], F16, tag="m2h", bufs=2)
                nc.vector.tensor_tensor(out=h[:], in0=ps2[:], in1=m2d[t_id][:],
                                        op=ALU.add)
                if DEBUG:
                    nc.sync.dma_start(
                        out=dbg_m2[128 * t_id:128 * (t_id + 1), :], in_=h[:])
                if t_id < 12:
                    nc.sync.dma_start(
                        out=m2_a[128 * t_id:128 * (t_id + 1), :], in_=h[:])
                else:
                    nc.sync.dma_start(
                        out=m2_b[128 * (t_id - 12):128 * (t_id - 11), :], in_=h[:])

            scatter(ag8_rows, idxs["1r"], sel1r_d, F8, C1R,
                    _bounds(nch1r_t), "ps_seg", 4, sfx="8", on_a=z_a, on_b=z_b,
                    preload_sel=True)

            # ---- gconv2: source-side partials over 1024 slots, transposed to
            #      [feat, slot] per slot tile, then ReduceScatter ----
            acc2 = [pmed.tile([128, 512], F16, tag="acc2", bufs=8, name=f"acc2{t}")
                    for t in range(8)]
            seg2a = scatter(m2_a, idxs["2a"], sel2a_d, F16, C2A,
                            _bounds(nch2a_t), "ps_seg", 2)
            for t_id, ps in seg2a:
                nc.vector.tensor_copy(out=acc2[t_id][:], in_=ps[:])

            def rs_tile(t_id, ps):
                pc = pmed.tile([128, 512], F16, tag="m2h", bufs=2)
                nc.vector.tensor_tensor(out=pc[:], in0=ps[:], in1=acc2[t_id][:],
                                        op=ALU.add)
                pst = pp.tile([128, 512], F16, tag="ps_t", bufs=2, space="PSUM")
                for k in range(4):
                    nc.tensor.transpose(
                        out=pst[:, 128 * k:128 * (k + 1)],
                        in_=pc[:, 128 * k:128 * (k + 1)], identity=ident[:])
                zb = pmed.tile([128, 512], F16, tag="zb", bufs=2)
                nc.scalar.activation(zb[:], pst[:], ACTF.Copy)
                nc.sync.dma_start(
                    out=rs_in[32 * t_id:32 * (t_id + 1), :].rearrange(
                        "a (b f) -> (a b) f", b=4),
                    in_=zb[:])

            scatter(m2_b, idxs["2b"], sel2b_d, F16, C2B,
                    _bounds(nch2b_t), "ps_seg", 2, on_a=rs_tile)
            nc.gpsimd.collective_compute(
                "ReduceScatter", ALU.add, replica_groups=RG,
                ins=[rs_in[:]], outs=[rs_out[:]])

            # ---- zk^T straight off the wire + readout ----
            zk16 = pmed.tile([128, 512], F16, tag="zb", bufs=2)
            nc.sync.dma_start(
                out=zk16[:],
                in_=rs_out[:].rearrange("a (b f) -> (a b) f", b=4))
            zkt = pmed.tile([128, 512], F16, tag="zkt", bufs=1)
            for k in range(4):
                kl = slice(128 * k, 128 * (k + 1))
                nc.scalar.activation(zkt[:, kl], zk16[:, kl], ACTF.Prelu,
                                     bias=bpf[:, 20 + k:21 + k], alpha=SLOPE)
            if DEBUG:
                zkf = pmed.tile([128, 512], F32, tag="zkf", bufs=1)
                for k in range(4):
                    ps = pp.tile([128, 512], F16, tag="ps_t", bufs=2, space="PSUM")
                    nc.tensor.transpose(out=ps[:, :128],
                                        in_=zkt[:, 128 * k:128 * (k + 1)],
                                        identity=ident[:])
                    nc.vector.tensor_copy(out=zkf[:, 128 * k:128 * (k + 1)],
                                          in_=ps[:, :128])
                nc.sync.dma_start(out=dbg_zk[:], in_=zkf[:])

            wg1 = [pw.tile([128, 512], F16, tag="wres", bufs=20, name=f"wg1{k}")
                   for k in range(4)]
            for k in range(4):
                nc.sync.dma_start(out=wg1[k][:], in_=gate_W1[128 * k:128 * (k + 1), :])
            w2r = psc.tile([128, 8], F16, tag="w2r", bufs=1)
            nc.sync.dma_start(out=w2r[:], in_=gw2reg[:])
            s1t = pmed.tile([128, 512], F16, tag="s1t", bufs=1)
            for ko_ in range(4):
                ps = pp.tile([128, 512], F32, tag="ps_mm", bufs=2, space="PSUM")
                for ki in range(4):
                    nc.tensor.matmul(ps[:, :128],
                                     lhsT=wg1[ki][:, 128 * ko_:128 * (ko_ + 1)],
                                     rhs=zkt[:, 128 * ki:128 * (ki + 1)],
                                     start=(ki == 0), stop=False)
                nc.tensor.matmul(ps[:, :128],
                                 lhsT=brow_g[:, 128 * ko_:128 * (ko_ + 1)],
                                 rhs=ones[:, :128], start=False, stop=True)
                nc.scalar.activation(s1t[:, 128 * ko_:128 * (ko_ + 1)], ps[:, :128],
                                     ACTF.Tanh)
            ps_sc = pp.tile([128, 512], F32, tag="ps_mm", bufs=2, space="PSUM")
            for ki in range(4):
                nc.tensor.matmul(ps_sc[:1, :128], lhsT=w2r[:, 2 * ki:2 * ki + 1],
                                 rhs=s1t[:, 128 * ki:128 * (ki + 1)],
                                 start=(ki == 0), stop=(ki == 3))
            ps_tr = pp.tile([128, 512], F32, tag="ps_seg", bufs=2, space="PSUM")
            for ki in range(4):
                nc.tensor.matmul(ps_tr[:1, :128], lhsT=w2r[:, 2 * ki + 1:2 * ki + 2],
                                 rhs=zkt[:, 128 * ki:128 * (ki + 1)],
                                 start=(ki == 0), stop=(ki == 3))
            erow = psc.tile([1, 128], F32, tag="erow", bufs=1)
            nc.scalar.activation(erow[:], ps_sc[:1, :128], ACTF.Exp,
                                 bias=bpf[:1, 24:25])
            etrow = psc.tile([1, 128], F32, tag="etrow", bufs=1)
            nc.vector.tensor_tensor(out=etrow[:], in0=erow[:], in1=ps_tr[:1, :128],
                                    op=ALU.mult)
            sums = psc.tile([1, 4], F32, tag="sums", bufs=1)
            nc.vector.tensor_reduce(out=sums[:, 0:2],
                                    in_=etrow[:].rearrange("p (g x) -> p g x", g=2),
                                    axis=AX, op=ALU.add)
            nc.vector.tensor_reduce(out=sums[:, 2:4],
                                    in_=erow[:].rearrange("p (g x) -> p g x", g=2),
                                    axis=AX, op=ALU.add)
            res = psc.tile([1, 4], F32, tag="res", bufs=1)
            nc.vector.reciprocal(out=res[:, 2:4], in_=sums[:, 2:4])
            nc.vector.tensor_tensor(out=res[:, 0:2], in0=sums[:, 0:2],
                                    in1=res[:, 2:4], op=ALU.mult)
            nc.vector.tensor_scalar(out=res[:, 0:2], in0=res[:, 0:2],
                                    scalar1=bpf[:1, 25:26], scalar2=None, op0=ALU.add)
            nc.sync.dma_start(out=out_d[:], in_=res[:, 0:2])

    nc.compile()
    return nc


def _ensure_ntff_hook():
    """Inject antenv.axon_hooks (absent in this image) so trace=True works."""
    import sys, types
    try:
        from antenv.axon_hooks import get_axon_ntff_profile_hook  # noqa
        return
    except ImportError:
        pass
    import antenv
    mod = types.ModuleType("antenv.axon_hooks")
    _state = {"hook": None}
    mod.set_axon_ntff_profile_hook = lambda h: _state.__setitem__("hook", h)
    mod.get_axon_ntff_profile_hook = lambda: _state["hook"]
    sys.modules["antenv.axon_hooks"] = mod
    antenv.axon_hooks = mod
    from trn_agent_boot.trn_boot import _ntff_profile_via_ctypes
    mod.set_axon_ntff_profile_hook(
        _ntff_profile_via_ctypes("/opt/axon/libaxon_pjrt.so"))


# ---------------------------------------------------------------------------
# host wrapper
# ---------------------------------------------------------------------------

def kernel(**inputs):
    f32 = lambda k: np.asarray(inputs[k], np.float32)
    x = f32("x"); pre_x = f32("pre_x")
    edge_index = np.asarray(inputs["edge_index"], np.int64)
    internal_edge_index = np.asarray(inputs["internal_edge_index"], np.int64)
    name_emb = f32("name_embeddings"); desc_emb = f32("desc_embeddings")
    ko_mask = np.asarray(inputs["ko_mask"], np.int64)
    bkm = np.asarray(inputs["batch_ko_masks"], np.int64)
    name_W = f32("name_W"); name_b = f32("name_b")
    desc_W = f32("desc_W"); desc_b = f32("desc_b")
    omic_W = f32("omic_W"); omic_b = f32("omic_b")
    fus_W = f32("fus_W"); fus_b = f32("fus_b")
    pre_W = f32("pre_W"); pre_b = f32("pre_b")
    ienc_W = f32("ienc_W"); ienc_b = f32("ienc_b")
    enc_W = f32("enc_W"); enc_b = f32("enc_b")
    gate_W1 = f32("gate_W1"); gate_b1 = f32("gate_b1")
    gate_W2 = f32("gate_W2"); gate_b2 = f32("gate_b2")
    reg_W = f32("reg_W"); reg_b = f32("reg_b")

    assert not fus_b.any() and not pre_b.any(), \
        "nonzero fus_b/pre_b not supported by this build"

    ko_feat = np.zeros(N, np.float32)
    ko_feat[ko_mask] = 1.0

    # ---- gconv2: source-sharded edges into the 1024 global KO slots ----
    slot_row = (bkm + np.arange(B)[:, None] * NE).reshape(-1)   # [1024]
    row2slots = {}
    for s_, r_ in enumerate(slot_row):
        row2slots.setdefault(int(r_), []).append(s_)
    def _pad_last(nch_t):
        nch_t[-1] += (-int(nch_t.sum())) % WAVE
        return tuple(int(v) for v in nch_t)

    s2_all, d2_all = edge_index[0], edge_index[1]
    m2mask = np.isin(d2_all, slot_row)
    per_core_2a = []   # sources in local rows [0, R/2)
    per_core_2b = []   # sources in local rows [R/2, R)
    needed = []        # per-core local rows whose z/m2 is actually consumed
    nch2a_t = np.ones(8, np.int64)
    nch2b_t = np.ones(8, np.int64)
    for c in range(NCORE):
        lo, hi = R * c, R * (c + 1)
        ss, ds = [], []
        for r_, sl_ in row2slots.items():
            if lo <= r_ < hi:
                for s_ in sl_:
                    ss.append(r_ - lo); ds.append(s_)
        mm = m2mask & (s2_all >= lo) & (s2_all < hi)
        for u, v in zip(s2_all[mm], d2_all[mm]):
            for s_ in row2slots[int(v)]:
                ss.append(int(u) - lo); ds.append(s_)
        src = np.array(ss, np.int64); dstl = np.array(ds, np.int64)
        nd = np.zeros(R, bool)
        nd[src] = True
        needed.append(nd)
        ha = src < 12 * 128
        per_core_2a.append((src[ha], dstl[ha]))
        per_core_2b.append((src[~ha] - 12 * 128, dstl[~ha]))
        nch2a_t = np.maximum(nch2a_t, -(-np.bincount(dstl[ha] >> 7, minlength=8) // 128))
        nch2b_t = np.maximum(nch2b_t, -(-np.bincount(dstl[~ha] >> 7, minlength=8) // 128))
    nch2a_t = _pad_last(nch2a_t)
    nch2b_t = _pad_last(nch2b_t)

    # ---- gconv1 edges (dst-sharded; self term added from local m1h).
    # Edges whose dst row never feeds gconv2 (not a slot row, not a source of
    # a slot edge) are dropped: their z rows are never read. Edges with a
    # LOCAL source are gathered from ag1_in during AG1. ----
    s1_all, d1_all = internal_edge_index[0], internal_edge_index[1]
    per_core_1l = []
    per_core_1r = []
    nch1l_t = np.ones(NT, np.int64)
    nch1r_t = np.ones(NT, np.int64)
    for c in range(NCORE):
        lo, hi = R * c, R * (c + 1)
        m = (d1_all >= lo) & (d1_all < hi)
        s1 = s1_all[m]; d1l = d1_all[m] - lo
        keep = needed[c][d1l]
        s1 = s1[keep]; d1l = d1l[keep]
        isloc = (s1 >= lo) & (s1 < hi)
        # the gconv self term rides the local pass as synthetic (r -> r) edges
        selfr = np.nonzero(needed[c])[0]
        ls = np.concatenate([s1[isloc] - lo, selfr])
        ld = np.concatenate([d1l[isloc], selfr])
        per_core_1l.append((ls, ld))
        per_core_1r.append((s1[~isloc], d1l[~isloc]))
        nch1l_t = np.maximum(
            nch1l_t, -(-np.bincount(ld >> 7, minlength=NT) // 128))
        nch1r_t = np.maximum(
            nch1r_t, -(-np.bincount(d1l[~isloc] >> 7, minlength=NT) // 128))
    nch1l_t = _pad_last(nch1l_t)
    nch1r_t = _pad_last(nch1r_t)

    nc = _build(nch1l_t, nch1r_t, nch2a_t, nch2b_t)

    import ml_dtypes
    f16 = np.float16
    f8 = ml_dtypes.float8_e4m3
    omic_Wp = _pad_w(omic_W, 512, 512)
    fus_ndp = _pad_w(fus_W[:2 * TX], 2 * TX, 512)
    fus_omp = _pad_w(fus_W[2 * TX:], 512, 512)
    wpe = pre_W @ enc_W                       # fold z-pre path into m2
    # [p, mo, ki, m] = W[ki*128+p, mo*128+m] so wstrip loads are contiguous
    name_Wr = np.ascontiguousarray(
        name_W.reshape(6, 128, 6, 128).transpose(1, 2, 0, 3).reshape(128, 6 * TX))
    desc_Wr = np.ascontiguousarray(
        desc_W.reshape(6, 128, 6, 128).transpose(1, 2, 0, 3).reshape(128, 6 * TX))
    fus_ndr = np.ascontiguousarray(
        fus_ndp.reshape(12, 128, 4, 128).transpose(1, 2, 0, 3).reshape(128, 6144))
    bias_pf = np.zeros((128, 26), np.float32)
    bias_pf[:, 0:6] = name_b.reshape(6, 128).T
    bias_pf[:, 6:12] = desc_b.reshape(6, 128).T
    bias_pf[:, 12:16] = _pad_w(omic_b[:, None], 512, 1).reshape(4, 128).T
    bias_pf[:, 16:20] = ienc_b.reshape(4, 128).T
    bias_pf[:, 20:24] = enc_b.reshape(4, 128).T
    bias_pf[:, 24] = float(gate_b2.reshape(-1)[0])
    bias_pf[:, 25] = float(reg_b.reshape(-1)[0])
    bias_rows = np.zeros((96, 512), np.float32)
    bias_rows[64, :] = gate_b1
    gw2 = np.concatenate([gate_W2, reg_W], axis=1).astype(np.float32)
    gw2 = np.ascontiguousarray(
        gw2.reshape(4, 128, 2).transpose(1, 0, 2).reshape(128, 8))

    shared = dict(
        name_W=name_Wr.astype(f16), desc_W=desc_Wr.astype(f16),
        omic_W=omic_Wp.astype(f16), fus_nd=fus_ndr.astype(f16),
        fus_om=fus_omp.astype(f16), ienc_W=ienc_W.astype(f16),
        wpe_W=wpe.astype(f16), enc_W=enc_W.astype(f16),
        gate_W1=gate_W1.astype(f16), gw2reg=gw2.astype(f16), bias_pf=bias_pf,
        bias_rows=bias_rows.astype(f16),
    )

    in_maps = []
    for c in range(NCORE):
        lo, hi = R * c, R * (c + 1)
        x_t = np.concatenate([x[lo:hi].T, ko_feat[None, lo:hi]], 0)
        pre_t = np.concatenate([pre_x[lo:hi].T, ko_feat[None, lo:hi]], 0)
        ndemb = np.concatenate(
            [name_emb[128 * c:128 * (c + 1)].T, desc_emb[128 * c:128 * (c + 1)].T], 0)
        ndemb = ndemb.reshape(12, 128, 128).transpose(1, 0, 2).reshape(128, 12 * 128)
        i1l, dv1l = _chunk_edges_per_tile(*per_core_1l[c], nch1l_t)
        i1r, dv1r = _chunk_edges_per_tile(*per_core_1r[c], nch1r_t)
        i2a, dv2a = _chunk_edges_per_tile(*per_core_2a[c], nch2a_t)
        i2b, dv2b = _chunk_edges_per_tile(*per_core_2b[c], nch2b_t)
        in_maps.append(dict(
            x_t=np.ascontiguousarray(x_t).astype(f16),
            pre_t=np.ascontiguousarray(pre_t).astype(f16),
            ndemb=np.ascontiguousarray(ndemb).astype(f16),
            idx1l=_wrap_idx_waves(i1l), sel1l=_sel_from_dstv(dv1l, f16),
            idx1r=_wrap_idx_waves(i1r), sel1r=_sel_from_dstv(dv1r, f8),
            idx2a=_wrap_idx_waves(i2a), sel2a=_sel_from_dstv(dv2a, f16),
            idx2b=_wrap_idx_waves(i2b), sel2b=_sel_from_dstv(dv2b, f16),
            **shared,
        ))

    if TRACE:
        _ensure_ntff_hook()
    res = run_bass_kernel_spmd(nc, in_maps, core_ids=list(range(NCORE)),
                               trace=TRACE, **(TRACE_KW or {}))
    kernel._last = res
    out = np.zeros(B, np.float32)
    for c in range(NCORE):
        out[2 * c:2 * c + 2] = res.results[c]["out"][0]
    return out


# revision 30
# speedup vs baseline: 1.0863x; 1.0240x over previous
"""Trainium2 Bass kernel for nn_MOTASG_KO_Reg (ragged graph-conv KO regression).

Strategy (8 NeuronCores, data-parallel over node rows):
  - N=16384 nodes = 16 batch samples x 1024 entities. Core c owns rows
    [2048c, 2048c+2048) = batch samples 2c, 2c+1.
  - Activations kept feature-major ("transposed", [feat, rows]) on chip so
    every linear is a native PE matmul (fp16 operands, fp32 PSUM).
  - name/desc path computed once on 128 entities/core, AllGathered, folded
    into cross INSIDE the fus PSUM chain via identity-inject matmuls.
  - z is never materialized: m2 = z @ enc_W is expanded as
      m2 = x_c @ enc_W + pre_c @ (pre_W @ enc_W) + lrelu(u) @ enc_W
    with pre_W @ enc_W precomputed on host. The dense part (m2d) runs
    during the fp8 AllGather window; the u part joins per tile after the
    gconv1 scatter.
  - gconv1 segment-sum via dma_gather + one-hot scatter matmuls in PSUM.
    Edges whose dst never feeds gconv2 are pruned (exact). Local-source
    edges (plus the self term as synthetic r->r edges) gather from fp16
    ag1_in DURING the AllGather; remote edges gather fp8 rows from the
    fp8 AllGather output.
  - All leaky-relus run on the scalar engine (ACTF.Prelu, PSUM input,
    per-partition bias AP, alpha=slope) - one op instead of three
    vector ops. Transposes run in fp16 (4x fp32 PE rate).
  - gconv2 source-side partials into the 1024 KO slots; partials are
    PE-transposed to [feat, slot] BEFORE the fp16 ReduceScatter so the
    readout (gate + softmax + weighted sum + regression) starts directly
    from rs_out with no transpose chain.
"""

import functools
import numpy as np

import concourse.bacc as bacc
import concourse.mybir as mybir
import concourse.tile as tile
from concourse import bass
from concourse.bass_utils import run_bass_kernel_spmd
from concourse.masks import make_identity

NE, B, KO = 1024, 16, 64
TX, OM, D = 768, 511, 512
N = NE * B
NCORE = 8
R = N // NCORE        # 2048 rows per core
NT = R // 128         # 16 row tiles per core
SLOPE = 0.3
F32 = mybir.dt.float32
F16 = mybir.dt.float16
F8 = mybir.dt.float8e4
I16 = mybir.dt.int16
AX = mybir.AxisListType.X
ALU = mybir.AluOpType
ACTF = mybir.ActivationFunctionType

WAVE = 8  # max gather chunks per dma_gather call
WCOLS = WAVE * 8


def _wave_sizes(C):
    """Two 4-chunk lead waves cut first-data latency; 8-chunk steady state."""
    return [4, 4] + [8] * ((C - 8) // 8)
DEBUG = False
PREP_TRIG = False
TRACE = False
TRACE_KW = None


# ---------------------------------------------------------------------------
# host-side edge preparation
# ---------------------------------------------------------------------------

def _chunk_edges_per_tile(src, dstl, nch_t):
    """Sort (src->dst_local) into per-destination-tile 128-edge chunks."""
    C = sum(nch_t)
    idx = np.zeros((C, 128), np.int16)
    dstv = np.full((C, 128), -2.0, np.float32)
    t_of = dstl >> 7
    base = 0
    for t, nch in enumerate(nch_t):
        m = t_of == t
        s = src[m]
        d = (dstl[m] - (t << 7)).astype(np.float32)
        n = len(s)
        assert n <= nch * 128, (n, nch)
        full, rem = divmod(n, 128)
        for j in range(full):
            idx[base + j] = s[j * 128:(j + 1) * 128]
            dstv[base + j] = d[j * 128:(j + 1) * 128]
        if rem:
            idx[base + full, :rem] = s[full * 128:]
            dstv[base + full, :rem] = d[full * 128:]
        base += nch
    return idx, dstv


def _wrap_idx_waves(idx_chunks):
    """[C,128] int16 -> [128, C*8] wrapped per dma_gather call."""
    C = idx_chunks.shape[0]
    cols = []
    cur = 0
    for s in _wave_sizes(C):
        lin = idx_chunks[cur:cur + s].reshape(-1)
        cur += s
        cols.append(np.tile(lin.reshape(-1, 16).T, (8, 1)))
    return np.ascontiguousarray(np.concatenate(cols, axis=1))


def _sel_from_dstv(dstv, dt):
    C = dstv.shape[0]
    sel = (dstv[:, :, None] == np.arange(128, dtype=np.float32)[None, None, :])
    sel = sel.astype(dt)          # [C, 128 slot, 128 dst]
    return np.ascontiguousarray(sel.transpose(1, 0, 2).reshape(128, C * 128))


def _pad_w(w, rows, cols):
    out = np.zeros((rows, cols), np.float32)
    out[:w.shape[0], :w.shape[1]] = w
    return out


# ---------------------------------------------------------------------------
# program builder
# ---------------------------------------------------------------------------

@functools.lru_cache(maxsize=4)
def _build(nch1l_t, nch1r_t, nch2a_t, nch2b_t, _flags=None):
    """gconv1 chunks per dst tile split into local-src (gathered from ag1_in
    during AG1) and remote-src sets; gconv2 chunks per slot tile split by
    m2-row half so its gathers overlap m2 production. Totals are multiples
    of WAVE."""
    C1L = sum(nch1l_t)
    C1R = sum(nch1r_t)
    C2A = sum(nch2a_t)
    C2B = sum(nch2b_t)
    nc = bacc.Bacc("TRN2", num_swdge_queues=4)

    def din(name, shape, dtype=F16):
        return nc.dram_tensor(name, shape, dtype, kind="ExternalInput")

    x_t = din("x_t", [512, R])                  # [x | ko]^T fp16
    pre_t_d = din("pre_t", [512, R])
    ndemb = din("ndemb", [128, 12 * 128])
    # pre-transposed on host: [p, mo, ki, m] = W[ki*128+p, mo*128+m]
    name_W = din("name_W", [128, 6 * TX])
    desc_W = din("desc_W", [128, 6 * TX])
    omic_W = din("omic_W", [512, 512])
    wnd_W = din("wnd_W", [128, 12 * 512])       # fus_nd @ ienc (host)
    wf_W = din("wf_W", [512, 512])              # fus_om @ ienc (host)
    wpe_W = din("wpe_W", [512, 512])            # pre_W @ enc_W (host)
    enc_W = din("enc_W", [512, 512])
    gate_W1 = din("gate_W1", [512, 512], F16)
    gw2reg = din("gw2reg", [128, 8], F16)
    bias_pf = din("bias_pf", [128, 26], F32)
    bias_rows = din("bias_rows", [96, 512], F16)
    idx1l_d = din("idx1l", [128, (C1L // WAVE) * WCOLS], I16)
    sel1l_d = din("sel1l", [128, C1L * 128], F16)
    idx1r_d = din("idx1r", [128, (C1R // WAVE) * WCOLS], I16)
    sel1r_d = din("sel1r", [128, C1R * 128], F8)
    idx2a_d = din("idx2a", [128, (C2A // WAVE) * WCOLS], I16)
    sel2a_d = din("sel2a", [128, C2A * 128], F16)
    idx2b_d = din("idx2b", [128, (C2B // WAVE) * WCOLS], I16)
    sel2b_d = din("sel2b", [128, C2B * 128], F16)
    out_d = nc.dram_tensor("out", [1, 2], F32, kind="ExternalOutput")

    agnd_in = nc.dram_tensor("agnd_in", [128, 512], F16)
    agnd_out = nc.dram_tensor("agnd_out", [NCORE * 128, 512], F16, addr_space="Shared")
    ag1_in = nc.dram_tensor("ag1_in", [R, 512], F16)
    ag8_in = nc.dram_tensor("ag8_in", [R // 4, 4 * 512], F8)
    ag8_out = nc.dram_tensor("ag8_out", [N // 4, 4 * 512], F8, addr_space="Shared")
    m2_a = nc.dram_tensor("m2_a", [12 * 128, 512], F16)
    m2_b = nc.dram_tensor("m2_b", [4 * 128, 512], F16)
    rs_in = nc.dram_tensor("rs_in", [NCORE * 32, 4 * 512], F16)
    rs_out = nc.dram_tensor("rs_out", [32, 4 * 512], F16)
    RG = [list(range(NCORE))]

    if DEBUG:
        dbg_m2 = nc.dram_tensor("dbg_m2", [R, 512], F16, kind="ExternalOutput")
        dbg_zk = nc.dram_tensor("dbg_zk", [128, 512], F32, kind="ExternalOutput")

    with tile.TileContext(nc) as tc:
        with (
            tc.tile_pool(name="pbig", bufs=8) as pbig,
            tc.tile_pool(name="ppre", bufs=4) as ppre,
            tc.tile_pool(name="pmed", bufs=1) as pmed,
            tc.tile_pool(name="pw", bufs=1) as pw,
            tc.tile_pool(name="pg", bufs=1) as pg,
            tc.tile_pool(name="psc", bufs=1) as psc,
            tc.tile_pool(name="pp", bufs=1, space="PSUM") as pp,
        ):
            # ---- ND embeddings first: one contiguous load feeding the
            # first matmuls ----
            emb_all = psc.tile([128, 12, 128], F16, tag="emb", bufs=1)
            nc.sync.dma_start(
                out=emb_all[:].rearrange("p a c -> p (a c)"), in_=ndemb[:])

            # ---- constants ----
            bpf = psc.tile([128, 26], F32, tag="bpf", bufs=1)
            nc.sync.dma_start(out=bpf[:], in_=bias_pf[:])
            brow_g = psc.tile([1, 512], F16, tag="brow_g", bufs=1)
            nc.sync.dma_start(out=brow_g[:], in_=bias_rows[64:65, :])
            ones = psc.tile([1, 512], F16, tag="ones", bufs=1)
            nc.vector.memset(ones[:], 1.0)
            ident = psc.tile([128, 128], F16, tag="ident", bufs=1)
            make_identity(nc, ident[:])
            idxs = {}
            for nm, dd in (("1l", idx1l_d), ("1r", idx1r_d),
                           ("2a", idx2a_d), ("2b", idx2b_d)):
                t_ = psc.tile([128, dd.shape[1]], I16, tag=f"idx{nm}", bufs=1)
                nc.sync.dma_start(out=t_[:], in_=dd[:])
                idxs[nm] = t_

            # ---- ND path (128 entities) — issued first so AG-nd fires early ----
            nd_act = []
            for half in range(2):
                W_d = name_W if half == 0 else desc_W
                embs = [emb_all[:, 6 * half + ki, :] for ki in range(6)]
                for mo in range(6):
                    ps = pp.tile([128, 512], F32, tag="ps_mm", bufs=2, space="PSUM")
                    wstrip = pw.tile([128, 6, 128], F16, tag="wnd6", bufs=2)
                    nc.sync.dma_start(
                        out=wstrip[:].rearrange("p a m -> p (a m)"),
                        in_=W_d[:, 768 * mo:768 * (mo + 1)])
                    for ki in range(6):
                        nc.tensor.matmul(ps[:, :128], lhsT=wstrip[:, ki, :],
                                         rhs=embs[ki],
                                         start=(ki == 0), stop=(ki == 5))
                    a = psc.tile([128, 128], F16, tag="ndact", bufs=12,
                                 name=f"ndact{half}_{mo}")
                    nc.scalar.activation(a[:], ps[:, :128], ACTF.Prelu,
                                         bias=bpf[:, 6 * half + mo:6 * half + mo + 1],
                                         alpha=SLOPE)
                    nd_act.append(a)
            # nd2 = [name_emb|desc_emb] @ (fus_nd @ ienc_W[:511]) for this
            # core's 128 entities, ROW-major [entity, 512] straight into AG
            wnds = []
            for ki in range(12):
                w_ = pw.tile([128, 512], F16, tag="wnd12", bufs=12)
                nc.sync.dma_start(out=w_[:], in_=wnd_W[:, 512 * ki:512 * (ki + 1)])
                wnds.append(w_)
            ps = pp.tile([128, 512], F32, tag="ps_mm", bufs=2, space="PSUM")
            for ki in range(12):
                nc.tensor.matmul(ps[:], lhsT=nd_act[ki][:], rhs=wnds[ki][:],
                                 start=(ki == 0), stop=(ki == 11))
            r_ = psc.tile([128, 512], F16, tag="ndres", bufs=1)
            nc.scalar.activation(r_[:], ps[:], ACTF.Copy)
            nc.sync.dma_start(out=agnd_in[:], in_=r_[:])
            nc.gpsimd.collective_compute(
                "AllGather", ALU.bypass, replica_groups=RG,
                ins=[agnd_in[:]], outs=[agnd_out[:]])

            # ---- weights first (small; unblock omic/m1/m2d early) ----
            womic = [pw.tile([128, 512], F16, tag="wres", bufs=16, name=f"womic{k}")
                     for k in range(4)]
            wf = [pw.tile([128, 512], F16, tag="wres", bufs=16, name=f"wf{k}")
                  for k in range(4)]
            wenc = [pw.tile([128, 512], F16, tag="wres", bufs=16, name=f"wenc{k}")
                    for k in range(4)]
            wpe = [pw.tile([128, 512], F16, tag="wres", bufs=16, name=f"wpe{k}")
                   for k in range(4)]
            for k in range(4):
                nc.sync.dma_start(out=womic[k][:], in_=omic_W[128 * k:128 * (k + 1), :])
                nc.sync.dma_start(out=wf[k][:], in_=wf_W[128 * k:128 * (k + 1), :])
                nc.sync.dma_start(out=wenc[k][:], in_=enc_W[128 * k:128 * (k + 1), :])
                nc.sync.dma_start(out=wpe[k][:], in_=wpe_W[128 * k:128 * (k + 1), :])
            ienc_ko = psc.tile([1, 512], F16, tag="ienc_ko", bufs=1)
            nc.sync.dma_start(out=ienc_ko[:], in_=bias_rows[65:66, :])
            ko_row = psc.tile([1, R], F16, tag="ko_row", bufs=1)
            nc.sync.dma_start(out=ko_row[:], in_=x_t[511:512, :])

            # ---- big activations (fp16) ----
            xt = []
            for k in range(4):
                t = pbig.tile([128, R], F16, tag="bigA", bufs=4, name=f"xt{k}")
                nc.sync.dma_start(out=t[:], in_=x_t[128 * k:128 * (k + 1), :])
                xt.append(t)
            pre = []
            for k in range(4):
                t = ppre.tile([128, R], F16, tag="pre", bufs=4, name=f"pre{k}")
                nc.sync.dma_start(out=t[:], in_=pre_t_d[128 * k:128 * (k + 1), :])
                pre.append(t)

            # nd2 tiles per entity block, ROW-major. Loaded via the gpsimd
            # (SWDGE) queue: they wait on AG-nd, and on the sync queue that
            # wait would head-of-line block the pre loads.
            nd_sb = [pmed.tile([128, 512], F16, tag="ndsb", bufs=8, name=f"ndsb{e}")
                     for e in range(8)]
            for e in range(8):
                nc.gpsimd.dma_start(out=nd_sb[e][:],
                                    in_=agnd_out[128 * e:128 * (e + 1), :])

            # ---- omic (all j): om_emb feature-major tiles ----
            om_all = {}
            for j in range(4):
                sl = slice(512 * j, 512 * (j + 1))
                for k in range(4):
                    ps = pp.tile([128, 512], F32, tag="ps_mm", bufs=2, space="PSUM")
                    for ki in range(4):
                        nc.tensor.matmul(ps[:], lhsT=womic[ki][:, 128 * k:128 * (k + 1)],
                                         rhs=xt[ki][:, sl], start=(ki == 0), stop=(ki == 3))
                    a = pmed.tile([128, 512], F16, tag="omj", bufs=16)
                    nc.scalar.activation(a[:], ps[:], ACTF.Prelu,
                                         bias=bpf[:, 12 + k:13 + k], alpha=SLOPE)
                    om_all[j, k] = a

            # ---- m1 = om_emb @ (fus_om @ ienc) + nd2[entities] + ko x ienc[511]
            # (cross never materializes; fus and m1 fused via host precompute)
            for t in range(NT):
                tsl = slice(128 * t, 128 * (t + 1))
                j, toff = t // 4, t % 4
                ps = pp.tile([128, 512], F32, tag="ps_seg", bufs=2, space="PSUM")
                for ki in range(4):
                    nc.tensor.matmul(ps[:],
                                     lhsT=om_all[j, ki][:, 128 * toff:128 * (toff + 1)],
                                     rhs=wf[ki][:], start=(ki == 0), stop=False)
                nc.tensor.matmul(ps[:], lhsT=ko_row[:, tsl], rhs=ienc_ko[:],
                                 start=False, stop=False)
                nc.tensor.matmul(ps[:], lhsT=ident[:], rhs=nd_sb[t % 8][:],
                                 start=False, stop=True)
                h = pmed.tile([128, 512], F16, tag="m1h", bufs=2)
                nc.vector.tensor_copy(out=h[:], in_=ps[:])
                nc.sync.dma_start(out=ag1_in[tsl, :], in_=h[:])
                h8 = pmed.tile([128, 512], F8, tag="m1h8", bufs=2)
                nc.scalar.activation(h8[:], ps[:], ACTF.Copy)
                nc.sync.dma_start(
                    out=ag8_in[32 * t:32 * (t + 1), :].rearrange(
                        "a (b f) -> (a b) f", b=4),
                    in_=h8[:])
            nc.gpsimd.collective_compute(
                "AllGather", ALU.bypass, replica_groups=RG,
                ins=[ag8_in[:]], outs=[ag8_out[:]])

            # ---- generic gather+scatter ----
            gsems = [nc.alloc_semaphore(f"gsem{q}") for q in range(4)]

            def _bounds(nch_t):
                b = []
                for t_id, nch in enumerate(nch_t):
                    for j in range(nch):
                        b.append((t_id, j == 0, j == nch - 1))
                return b

            def scatter(src_dram, idx_t, sel_d, sel_dt, nchunks, tile_bounds,
                        psum_tag, gbufs_n, sfx="", on_a=None, on_b=None,
                        preload_sel=False, sel_sfx=None):
                """Two-phase pipelined per-tile post-processing: on_a(t, ps)
                fires one tile late (at the next tile's first chunk) and
                returns a context; on_b(ctx) fires another tile later. The
                delay keeps the tensor queue from stalling on cross-engine
                dependencies of the post-processing. preload_sel issues every
                sel-wave DMA up front so later sel loads are never head-of-line
                blocked behind result DMAs on the sync queue."""
                out_psums = []
                qa, qb = [], []
                ps = None
                src_ap = src_dram if isinstance(src_dram, bass.AP) else src_dram[:]
                wsz = _wave_sizes(nchunks)
                ssfx = sfx if sel_sfx is None else sel_sfx

                sels = []
                if preload_sel:
                    cur = 0
                    for w, s in enumerate(wsz):
                        sw = pg.tile([128, WAVE, 128], sel_dt, tag="selw" + ssfx,
                                     bufs=len(wsz))
                        nc.sync.dma_start(
                            out=sw[:, :s, :].rearrange("p a d -> p (a d)"),
                            in_=sel_d[:, 128 * cur:128 * (cur + s)])
                        sels.append(sw)
                        cur += s

                def pump():
                    if qb and on_b is not None:
                        on_b(qb.pop(0))
                    if qa:
                        t_id_, ps_ = qa.pop(0)
                        if on_a is not None:
                            ctx = on_a(t_id_, ps_)
                            if on_b is not None:
                                qb.append(ctx)
                cur = 0
                i = 0
                for w, s in enumerate(wsz):
                    g = pg.tile([128, WAVE, 512], sel_dt, tag="gath" + sfx,
                                bufs=gbufs_n)
                    # prepare_only: descriptor gen is the only gpsimd work; the
                    # transfer runs on SDMA after the (cheap) trigger, so waves
                    # on different queues genuinely overlap.
                    q = w % 4
                    if PREP_TRIG:
                        nc.gpsimd.dma_gather(
                            g[:, :s, :], src_ap, idx_t[:, 8 * cur:8 * (cur + s)],
                            s * 128, s * 128, 512,
                            single_packet=True, queue_num=q,
                            prepare_only=True, sem=gsems[q])
                        nc.gpsimd.trigger_dma(count=None, queue_num=q)
                    else:
                        nc.gpsimd.dma_gather(
                            g[:, :s, :], src_ap, idx_t[:, 8 * cur:8 * (cur + s)],
                            s * 128, s * 128, 512,
                            single_packet=True, queue_num=q)
                    if preload_sel:
                        sw = sels[w]
                    else:
                        sw = pg.tile([128, WAVE, 128], sel_dt, tag="selw" + ssfx,
                                     bufs=gbufs_n)
                        nc.sync.dma_start(
                            out=sw[:, :s, :].rearrange("p a d -> p (a d)"),
                            in_=sel_d[:, 128 * cur:128 * (cur + s)])
                    for slot in range(s):
                        t_id, first, last = tile_bounds[i]
                        i += 1
                        if first:
                            pump()
                            ps = pp.tile([128, 512], F32, tag=psum_tag,
                                         bufs=2, space="PSUM")
                        nc.tensor.matmul(ps[:], lhsT=sw[:, slot, :],
                                         rhs=g[:, slot, :],
                                         start=first, stop=last)
                        if last:
                            out_psums.append((t_id, ps))
                            qa.append((t_id, ps))
                    cur += s
                pump()
                pump()
                return out_psums

            # ---- gconv1 local-src edges: gathered from ag1_in DURING AG1.
            # Emitted BEFORE m2d so the tensor queue consumes the 1l waves as
            # they land (g double-buffer recycles; gathers never stall). ----
            uacc = [pmed.tile([128, 512], F16, tag="uacc", bufs=NT, name=f"uacc{t}")
                    for t in range(NT)]
            seg1l = scatter(ag1_in, idxs["1l"], sel1l_d, F16, C1L,
                            _bounds(nch1l_t), "ps_seg", 2)
            for t_id, ps in seg1l:
                nc.vector.tensor_copy(out=uacc[t_id][:], in_=ps[:])

            # ---- m2 dense part fills the rest of the AG window:
            #      m2d = x_c @ enc_W + pre_c @ (pre_W @ enc_W) ----
            m2d = [pmed.tile([128, 512], F16, tag="m2d", bufs=NT, name=f"m2d{t}")
                   for t in range(NT)]
            for t in range(NT):
                tsl = slice(128 * t, 128 * (t + 1))
                ps = pp.tile([128, 512], F32, tag="ps_mm", bufs=2, space="PSUM")
                for ki in range(4):
                    nc.tensor.matmul(ps[:], lhsT=xt[ki][:, tsl], rhs=wenc[ki][:],
                                     start=(ki == 0), stop=False)
                for ki in range(4):
                    nc.tensor.matmul(ps[:], lhsT=pre[ki][:, tsl], rhs=wpe[ki][:],
                                     start=False, stop=(ki == 3))
                nc.scalar.activation(m2d[t][:], ps[:], ACTF.Copy)

            # ---- gconv1 remote edges; per tile: u -> lrelu(u)^T strips ->
            #      m2 = m2d + lrelu(u) @ enc_W -> DRAM for gconv2 ----
            ag8_rows = ag8_out[:].rearrange("a (b f) -> (a b) f", b=4)

            def z_a(t_id, ps):
                useg = pmed.tile([128, 512], F16, tag="useg", bufs=2)
                nc.vector.tensor_tensor(out=useg[:], in0=ps[:],
                                        in1=uacc[t_id][:], op=ALU.add)
                pst = pp.tile([128, 512], F16, tag="ps_t", bufs=2, space="PSUM")
                for k in range(4):
                    nc.tensor.transpose(
                        out=pst[:, 128 * k:128 * (k + 1)],
                        in_=useg[:, 128 * k:128 * (k + 1)], identity=ident[:])
                ut = pmed.tile([128, 512], F16, tag="ut", bufs=2)
                for k in range(4):
                    nc.scalar.activation(ut[:, 128 * k:128 * (k + 1)],
                                         pst[:, 128 * k:128 * (k + 1)], ACTF.Prelu,
                                         bias=bpf[:, 16 + k:17 + k], alpha=SLOPE)
                return (t_id, ut)

            def z_b(ctx):
                t_id, ut = ctx
                ps2 = pp.tile([128, 512], F32, tag="ps_m2", bufs=2, space="PSUM")
                for ki in range(4):
                    nc.tensor.matmul(ps2[:], lhsT=ut[:, 128 * ki:128 * (ki + 1)],
                                     rhs=wenc[ki][:], start=(ki == 0), stop=(ki == 3))
                h = pmed.tile([128, 512], F16, tag="m2h", bufs=2)
                nc.vector.tensor_tensor(out=h[:], in0=ps2[:], in1=m2d[t_id][:],
                                        op=ALU.add)
                if DEBUG:
                    nc.sync.dma_start(
                        out=dbg_m2[128 * t_id:128 * (t_id + 1), :], in_=h[:])
                if t_id < 12:
                    nc.sync.dma_start(
                        out=m2_a[128 * t_id:128 * (t_id + 1), :], in_=h[:])
                else:
                    nc.sync.dma_start(
                        out=m2_b[128 * (t_id - 12):128 * (t_id - 11), :], in_=h[:])

            scatter(ag8_rows, idxs["1r"], sel1r_d, F8, C1R,
                    _bounds(nch1r_t), "ps_seg", 3, sfx="8", on_a=z_a, on_b=z_b,
                    preload_sel=True)

            # ---- gconv2: source-side partials over 1024 slots, transposed to
            #      [feat, slot] per slot tile, then ReduceScatter ----
            acc2 = [pmed.tile([128, 512], F16, tag="acc2", bufs=8, name=f"acc2{t}")
                    for t in range(8)]
            seg2a = scatter(m2_a, idxs["2a"], sel2a_d, F16, C2A,
                            _bounds(nch2a_t), "ps_seg", 2)
            for t_id, ps in seg2a:
                nc.vector.tensor_copy(out=acc2[t_id][:], in_=ps[:])

            def rs_tile(t_id, ps):
                pc = pmed.tile([128, 512], F16, tag="m2h", bufs=2)
                nc.vector.tensor_tensor(out=pc[:], in0=ps[:], in1=acc2[t_id][:],
                                        op=ALU.add)
                pst = pp.tile([128, 512], F16, tag="ps_t", bufs=2, space="PSUM")
                for k in range(4):
                    nc.tensor.transpose(
                        out=pst[:, 128 * k:128 * (k + 1)],
                        in_=pc[:, 128 * k:128 * (k + 1)], identity=ident[:])
                zb = pmed.tile([128, 512], F16, tag="zb", bufs=2)
                nc.scalar.activation(zb[:], pst[:], ACTF.Copy)
                nc.sync.dma_start(
                    out=rs_in[32 * t_id:32 * (t_id + 1), :].rearrange(
                        "a (b f) -> (a b) f", b=4),
                    in_=zb[:])

            scatter(m2_b, idxs["2b"], sel2b_d, F16, C2B,
                    _bounds(nch2b_t), "ps_seg", 2, on_a=rs_tile,
                    preload_sel=True, sel_sfx="2b")
            nc.gpsimd.collective_compute(
                "ReduceScatter", ALU.add, replica_groups=RG,
                ins=[rs_in[:]], outs=[rs_out[:]])

            # ---- zk^T straight off the wire + readout ----
            zk16 = pmed.tile([128, 512], F16, tag="zb", bufs=2)
            nc.sync.dma_start(
                out=zk16[:],
                in_=rs_out[:].rearrange("a (b f) -> (a b) f", b=4))
            zkt = pmed.tile([128, 512], F16, tag="zkt", bufs=1)
            for k in range(4):
                kl = slice(128 * k, 128 * (k + 1))
                nc.scalar.activation(zkt[:, kl], zk16[:, kl], ACTF.Prelu,
                                     bias=bpf[:, 20 + k:21 + k], alpha=SLOPE)
            if DEBUG:
                zkf = pmed.tile([128, 512], F32, tag="zkf", bufs=1)
                for k in range(4):
                    ps = pp.tile([128, 512], F16, tag="ps_t", bufs=2, space="PSUM")
                    nc.tensor.transpose(out=ps[:, :128],
                                        in_=zkt[:, 128 * k:128 * (k + 1)],
                                        identity=ident[:])
                    nc.vector.tensor_copy(out=zkf[:, 128 * k:128 * (k + 1)],
                                          in_=ps[:, :128])
                nc.sync.dma_start(out=dbg_zk[:], in_=zkf[:])

            wg1 = [pw.tile([128, 512], F16, tag="wres", bufs=16, name=f"wg1{k}")
                   for k in range(4)]
            for k in range(4):
                nc.sync.dma_start(out=wg1[k][:], in_=gate_W1[128 * k:128 * (k + 1), :])
            w2r = psc.tile([128, 8], F16, tag="w2r", bufs=1)
            nc.sync.dma_start(out=w2r[:], in_=gw2reg[:])
            s1t = pmed.tile([128, 512], F16, tag="s1t", bufs=1)
            for ko_ in range(4):
                ps = pp.tile([128, 512], F32, tag="ps_mm", bufs=2, space="PSUM")
                for ki in range(4):
                    nc.tensor.matmul(ps[:, :128],
                                     lhsT=wg1[ki][:, 128 * ko_:128 * (ko_ + 1)],
                                     rhs=zkt[:, 128 * ki:128 * (ki + 1)],
                                     start=(ki == 0), stop=False)
                nc.tensor.matmul(ps[:, :128],
                                 lhsT=brow_g[:, 128 * ko_:128 * (ko_ + 1)],
                                 rhs=ones[:, :128], start=False, stop=True)
                nc.scalar.activation(s1t[:, 128 * ko_:128 * (ko_ + 1)], ps[:, :128],
                                     ACTF.Tanh)
            ps_sc = pp.tile([128, 512], F32, tag="ps_mm", bufs=2, space="PSUM")
            for ki in range(4):
                nc.tensor.matmul(ps_sc[:1, :128], lhsT=w2r[:, 2 * ki:2 * ki + 1],
                                 rhs=s1t[:, 128 * ki:128 * (ki + 1)],
                                 start=(ki == 0), stop=(ki == 3))
            ps_tr = pp.tile([128, 512], F32, tag="ps_seg", bufs=2, space="PSUM")
            for ki in range(4):
                nc.tensor.matmul(ps_tr[:1, :128], lhsT=w2r[:, 2 * ki + 1:2 * ki + 2],
                                 rhs=zkt[:, 128 * ki:128 * (ki + 1)],
                                 start=(ki == 0), stop=(ki == 3))
            erow = psc.tile([1, 128], F32, tag="erow", bufs=1)
            nc.scalar.activation(erow[:], ps_sc[:1, :128], ACTF.Exp,
                                 bias=bpf[:1, 24:25])
            etrow = psc.tile([1, 128], F32, tag="etrow", bufs=1)
            nc.vector.tensor_tensor(out=etrow[:], in0=erow[:], in1=ps_tr[:1, :128],
                                    op=ALU.mult)
            sums = psc.tile([1, 4], F32, tag="sums", bufs=1)
            nc.vector.tensor_reduce(out=sums[:, 0:2],
                                    in_=etrow[:].rearrange("p (g x) -> p g x", g=2),
                                    axis=AX, op=ALU.add)
            nc.vector.tensor_reduce(out=sums[:, 2:4],
                                    in_=erow[:].rearrange("p (g x) -> p g x", g=2),
                                    axis=AX, op=ALU.add)
            res = psc.tile([1, 4], F32, tag="res", bufs=1)
            nc.vector.reciprocal(out=res[:, 2:4], in_=sums[:, 2:4])
            nc.vector.tensor_tensor(out=res[:, 0:2], in0=sums[:, 0:2],
                                    in1=res[:, 2:4], op=ALU.mult)
            nc.vector.tensor_scalar(out=res[:, 0:2], in0=res[:, 0:2],
                                    scalar1=bpf[:1, 25:26], scalar2=None, op0=ALU.add)
            nc.sync.dma_start(out=out_d[:], in_=res[:, 0:2])

    nc.compile()
    return nc


def _ensure_ntff_hook():
    """Inject antenv.axon_hooks (absent in this image) so trace=True works."""
    import sys, types
    try:
        from antenv.axon_hooks import get_axon_ntff_profile_hook  # noqa
        return
    except ImportError:
        pass
    import antenv
    mod = types.ModuleType("antenv.axon_hooks")
    _state = {"hook": None}
    mod.set_axon_ntff_profile_hook = lambda h: _state.__setitem__("hook", h)
    mod.get_axon_ntff_profile_hook = lambda: _state["hook"]
    sys.modules["antenv.axon_hooks"] = mod
    antenv.axon_hooks = mod
    from trn_agent_boot.trn_boot import _ntff_profile_via_ctypes
    mod.set_axon_ntff_profile_hook(
        _ntff_profile_via_ctypes("/opt/axon/libaxon_pjrt.so"))


# ---------------------------------------------------------------------------
# host wrapper
# ---------------------------------------------------------------------------

def kernel(**inputs):
    f32 = lambda k: np.asarray(inputs[k], np.float32)
    x = f32("x"); pre_x = f32("pre_x")
    edge_index = np.asarray(inputs["edge_index"], np.int64)
    internal_edge_index = np.asarray(inputs["internal_edge_index"], np.int64)
    name_emb = f32("name_embeddings"); desc_emb = f32("desc_embeddings")
    ko_mask = np.asarray(inputs["ko_mask"], np.int64)
    bkm = np.asarray(inputs["batch_ko_masks"], np.int64)
    name_W = f32("name_W"); name_b = f32("name_b")
    desc_W = f32("desc_W"); desc_b = f32("desc_b")
    omic_W = f32("omic_W"); omic_b = f32("omic_b")
    fus_W = f32("fus_W"); fus_b = f32("fus_b")
    pre_W = f32("pre_W"); pre_b = f32("pre_b")
    ienc_W = f32("ienc_W"); ienc_b = f32("ienc_b")
    enc_W = f32("enc_W"); enc_b = f32("enc_b")
    gate_W1 = f32("gate_W1"); gate_b1 = f32("gate_b1")
    gate_W2 = f32("gate_W2"); gate_b2 = f32("gate_b2")
    reg_W = f32("reg_W"); reg_b = f32("reg_b")

    assert not fus_b.any() and not pre_b.any(), \
        "nonzero fus_b/pre_b not supported by this build"

    ko_feat = np.zeros(N, np.float32)
    ko_feat[ko_mask] = 1.0

    # ---- gconv2: source-sharded edges into the 1024 global KO slots ----
    slot_row = (bkm + np.arange(B)[:, None] * NE).reshape(-1)   # [1024]
    row2slots = {}
    for s_, r_ in enumerate(slot_row):
        row2slots.setdefault(int(r_), []).append(s_)
    def _pad_last(nch_t):
        nch_t[-1] += (-int(nch_t.sum())) % WAVE
        return tuple(int(v) for v in nch_t)

    s2_all, d2_all = edge_index[0], edge_index[1]
    m2mask = np.isin(d2_all, slot_row)
    per_core_2a = []   # sources in local rows [0, R/2)
    per_core_2b = []   # sources in local rows [R/2, R)
    needed = []        # per-core local rows whose z/m2 is actually consumed
    nch2a_t = np.ones(8, np.int64)
    nch2b_t = np.ones(8, np.int64)
    for c in range(NCORE):
        lo, hi = R * c, R * (c + 1)
        ss, ds = [], []
        for r_, sl_ in row2slots.items():
            if lo <= r_ < hi:
                for s_ in sl_:
                    ss.append(r_ - lo); ds.append(s_)
        mm = m2mask & (s2_all >= lo) & (s2_all < hi)
        for u, v in zip(s2_all[mm], d2_all[mm]):
            for s_ in row2slots[int(v)]:
                ss.append(int(u) - lo); ds.append(s_)
        src = np.array(ss, np.int64); dstl = np.array(ds, np.int64)
        nd = np.zeros(R, bool)
        nd[src] = True
        needed.append(nd)
        ha = src < 12 * 128
        per_core_2a.append((src[ha], dstl[ha]))
        per_core_2b.append((src[~ha] - 12 * 128, dstl[~ha]))
        nch2a_t = np.maximum(nch2a_t, -(-np.bincount(dstl[ha] >> 7, minlength=8) // 128))
        nch2b_t = np.maximum(nch2b_t, -(-np.bincount(dstl[~ha] >> 7, minlength=8) // 128))
    nch2a_t = _pad_last(nch2a_t)
    nch2b_t = _pad_last(nch2b_t)

    # ---- gconv1 edges (dst-sharded; self term added from local m1h).
    # Edges whose dst row never feeds gconv2 (not a slot row, not a source of
    # a slot edge) are dropped: their z rows are never read. Edges with a
    # LOCAL source are gathered from ag1_in during AG1. ----
    s1_all, d1_all = internal_edge_index[0], internal_edge_index[1]
    per_core_1l = []
    per_core_1r = []
    nch1l_t = np.ones(NT, np.int64)
    nch1r_t = np.ones(NT, np.int64)
    for c in range(NCORE):
        lo, hi = R * c, R * (c + 1)
        m = (d1_all >= lo) & (d1_all < hi)
        s1 = s1_all[m]; d1l = d1_all[m] - lo
        keep = needed[c][d1l]
        s1 = s1[keep]; d1l = d1l[keep]
        isloc = (s1 >= lo) & (s1 < hi)
        # the gconv self term rides the local pass as synthetic (r -> r) edges
        selfr = np.nonzero(needed[c])[0]
        ls = np.concatenate([s1[isloc] - lo, selfr])
        ld = np.concatenate([d1l[isloc], selfr])
        per_core_1l.append((ls, ld))
        per_core_1r.append((s1[~isloc], d1l[~isloc]))
        nch1l_t = np.maximum(
            nch1l_t, -(-np.bincount(ld >> 7, minlength=NT) // 128))
        nch1r_t = np.maximum(
            nch1r_t, -(-np.bincount(d1l[~isloc] >> 7, minlength=NT) // 128))
    nch1l_t = _pad_last(nch1l_t)
    nch1r_t = _pad_last(nch1r_t)

    nc = _build(nch1l_t, nch1r_t, nch2a_t, nch2b_t, (DEBUG, PREP_TRIG))

    import ml_dtypes
    f16 = np.float16
    f8 = ml_dtypes.float8_e4m3
    omic_Wp = _pad_w(omic_W, 512, 512)
    fus_ndp = _pad_w(fus_W[:2 * TX], 2 * TX, 512)
    fus_omp = _pad_w(fus_W[2 * TX:], 512, 512)
    wpe = pre_W @ enc_W                       # fold z-pre path into m2
    wf = fus_omp[:, :OM] @ ienc_W[:OM, :]     # fuse fus_om into m1
    wnd = fus_ndp[:, :OM] @ ienc_W[:OM, :]    # fuse fus_nd into nd2
    wnd_r = np.ascontiguousarray(
        wnd.reshape(12, 128, 512).transpose(1, 0, 2).reshape(128, 12 * 512))
    # [p, mo, ki, m] = W[ki*128+p, mo*128+m] so wstrip loads are contiguous
    name_Wr = np.ascontiguousarray(
        name_W.reshape(6, 128, 6, 128).transpose(1, 2, 0, 3).reshape(128, 6 * TX))
    desc_Wr = np.ascontiguousarray(
        desc_W.reshape(6, 128, 6, 128).transpose(1, 2, 0, 3).reshape(128, 6 * TX))
    bias_pf = np.zeros((128, 26), np.float32)
    bias_pf[:, 0:6] = name_b.reshape(6, 128).T
    bias_pf[:, 6:12] = desc_b.reshape(6, 128).T
    bias_pf[:, 12:16] = _pad_w(omic_b[:, None], 512, 1).reshape(4, 128).T
    bias_pf[:, 16:20] = ienc_b.reshape(4, 128).T
    bias_pf[:, 20:24] = enc_b.reshape(4, 128).T
    bias_pf[:, 24] = float(gate_b2.reshape(-1)[0])
    bias_pf[:, 25] = float(reg_b.reshape(-1)[0])
    bias_rows = np.zeros((96, 512), np.float32)
    bias_rows[64, :] = gate_b1
    bias_rows[65, :] = ienc_W[OM, :]
    gw2 = np.concatenate([gate_W2, reg_W], axis=1).astype(np.float32)
    gw2 = np.ascontiguousarray(
        gw2.reshape(4, 128, 2).transpose(1, 0, 2).reshape(128, 8))

    shared = dict(
        name_W=name_Wr.astype(f16), desc_W=desc_Wr.astype(f16),
        omic_W=omic_Wp.astype(f16), wnd_W=wnd_r.astype(f16),
        wf_W=wf.astype(f16),
        wpe_W=wpe.astype(f16), enc_W=enc_W.astype(f16),
        gate_W1=gate_W1.astype(f16), gw2reg=gw2.astype(f16), bias_pf=bias_pf,
        bias_rows=bias_rows.astype(f16),
    )

    in_maps = []
    for c in range(NCORE):
        lo, hi = R * c, R * (c + 1)
        x_t = np.concatenate([x[lo:hi].T, ko_feat[None, lo:hi]], 0)
        pre_t = np.concatenate([pre_x[lo:hi].T, ko_feat[None, lo:hi]], 0)
        ndemb = np.concatenate(
            [name_emb[128 * c:128 * (c + 1)].T, desc_emb[128 * c:128 * (c + 1)].T], 0)
        ndemb = ndemb.reshape(12, 128, 128).transpose(1, 0, 2).reshape(128, 12 * 128)
        i1l, dv1l = _chunk_edges_per_tile(*per_core_1l[c], nch1l_t)
        i1r, dv1r = _chunk_edges_per_tile(*per_core_1r[c], nch1r_t)
        i2a, dv2a = _chunk_edges_per_tile(*per_core_2a[c], nch2a_t)
        i2b, dv2b = _chunk_edges_per_tile(*per_core_2b[c], nch2b_t)
        in_maps.append(dict(
            x_t=np.ascontiguousarray(x_t).astype(f16),
            pre_t=np.ascontiguousarray(pre_t).astype(f16),
            ndemb=np.ascontiguousarray(ndemb).astype(f16),
            idx1l=_wrap_idx_waves(i1l), sel1l=_sel_from_dstv(dv1l, f16),
            idx1r=_wrap_idx_waves(i1r), sel1r=_sel_from_dstv(dv1r, f8),
            idx2a=_wrap_idx_waves(i2a), sel2a=_sel_from_dstv(dv2a, f16),
            idx2b=_wrap_idx_waves(i2b), sel2b=_sel_from_dstv(dv2b, f16),
            **shared,
        ))

    if TRACE:
        _ensure_ntff_hook()
    res = run_bass_kernel_spmd(nc, in_maps, core_ids=list(range(NCORE)),
                               trace=TRACE, **(TRACE_KW or {}))
    kernel._last = res
    out = np.zeros(B, np.float32)
    for c in range(NCORE):
        out[2 * c:2 * c + 2] = res.results[c]["out"][0]
    return out


# revision 33
# speedup vs baseline: 1.1770x; 1.0834x over previous
"""Trainium2 Bass kernel for nn_MOTASG_KO_Reg (ragged graph-conv KO regression).

Strategy (8 NeuronCores, data-parallel over node rows):
  - N=16384 nodes = 16 batch samples x 1024 entities. Core c owns rows
    [2048c, 2048c+2048) = batch samples 2c, 2c+1.
  - Activations kept feature-major ("transposed", [feat, rows]) on chip so
    every linear is a native PE matmul (fp16 operands, fp32 PSUM).
  - name/desc path computed once on 128 entities/core, AllGathered, folded
    into cross INSIDE the fus PSUM chain via identity-inject matmuls.
  - z is never materialized: m2 = z @ enc_W is expanded as
      m2 = x_c @ enc_W + pre_c @ (pre_W @ enc_W) + lrelu(u) @ enc_W
    with pre_W @ enc_W precomputed on host. The dense part (m2d) runs
    during the fp8 AllGather window; the u part joins per tile after the
    gconv1 scatter.
  - gconv1 segment-sum via dma_gather + one-hot scatter matmuls in PSUM.
    Edges whose dst never feeds gconv2 are pruned (exact). Local-source
    edges (plus the self term as synthetic r->r edges) gather from fp16
    ag1_in DURING the AllGather; remote edges gather fp8 rows from the
    fp8 AllGather output.
  - All leaky-relus run on the scalar engine (ACTF.Prelu, PSUM input,
    per-partition bias AP, alpha=slope) - one op instead of three
    vector ops. Transposes run in fp16 (4x fp32 PE rate).
  - gconv2 source-side partials into the 1024 KO slots; partials are
    PE-transposed to [feat, slot] BEFORE the fp16 ReduceScatter so the
    readout (gate + softmax + weighted sum + regression) starts directly
    from rs_out with no transpose chain.
"""

import functools
import numpy as np

import concourse.bacc as bacc
import concourse.mybir as mybir
import concourse.tile as tile
from concourse import bass
from concourse.bass_utils import run_bass_kernel_spmd
from concourse.masks import make_identity

NE, B, KO = 1024, 16, 64
TX, OM, D = 768, 511, 512
N = NE * B
NCORE = 8
R = N // NCORE        # 2048 rows per core
NT = R // 128         # 16 row tiles per core
SLOPE = 0.3
F32 = mybir.dt.float32
F16 = mybir.dt.float16
F8 = mybir.dt.float8e4
I16 = mybir.dt.int16
AX = mybir.AxisListType.X
ALU = mybir.AluOpType
ACTF = mybir.ActivationFunctionType

WAVE = 8  # max gather chunks per dma_gather call
WCOLS = WAVE * 8


def _wave_sizes(C):
    """Two 4-chunk lead waves cut first-data latency; 8-chunk steady state."""
    return [4, 4] + [8] * ((C - 8) // 8)
DEBUG = False
PREP_TRIG = False
TRACE = False
TRACE_KW = None


# ---------------------------------------------------------------------------
# host-side edge preparation
# ---------------------------------------------------------------------------

def _chunk_edges_per_tile(src, dstl, nch_t):
    """Sort (src->dst_local) into per-destination-tile 128-edge chunks."""
    C = sum(nch_t)
    idx = np.zeros((C, 128), np.int16)
    dstv = np.full((C, 128), -2.0, np.float32)
    t_of = dstl >> 7
    base = 0
    for t, nch in enumerate(nch_t):
        m = t_of == t
        s = src[m]
        d = (dstl[m] - (t << 7)).astype(np.float32)
        n = len(s)
        assert n <= nch * 128, (n, nch)
        full, rem = divmod(n, 128)
        for j in range(full):
            idx[base + j] = s[j * 128:(j + 1) * 128]
            dstv[base + j] = d[j * 128:(j + 1) * 128]
        if rem:
            idx[base + full, :rem] = s[full * 128:]
            dstv[base + full, :rem] = d[full * 128:]
        base += nch
    return idx, dstv


def _wrap_idx_waves(idx_chunks):
    """[C,128] int16 -> [128, C*8] wrapped per dma_gather call."""
    C = idx_chunks.shape[0]
    cols = []
    cur = 0
    for s in _wave_sizes(C):
        lin = idx_chunks[cur:cur + s].reshape(-1)
        cur += s
        cols.append(np.tile(lin.reshape(-1, 16).T, (8, 1)))
    return np.ascontiguousarray(np.concatenate(cols, axis=1))


def _sel_from_dstv(dstv, dt):
    C = dstv.shape[0]
    sel = (dstv[:, :, None] == np.arange(128, dtype=np.float32)[None, None, :])
    sel = sel.astype(dt)          # [C, 128 slot, 128 dst]
    return np.ascontiguousarray(sel.transpose(1, 0, 2).reshape(128, C * 128))


def _pad_w(w, rows, cols):
    out = np.zeros((rows, cols), np.float32)
    out[:w.shape[0], :w.shape[1]] = w
    return out


# ---------------------------------------------------------------------------
# program builder
# ---------------------------------------------------------------------------

@functools.lru_cache(maxsize=4)
def _build(nch1l_t, nch1r_t, nch2a_t, nch2b_t, _flags=None):
    """gconv1 chunks per dst tile split into local-src (gathered from ag1_in
    during AG1) and remote-src sets; gconv2 chunks per slot tile split by
    m2-row half so its gathers overlap m2 production. Totals are multiples
    of WAVE."""
    C1L = sum(nch1l_t)
    C1R = sum(nch1r_t)
    C2A = sum(nch2a_t)
    C2B = sum(nch2b_t)
    nc = bacc.Bacc("TRN2", num_swdge_queues=4)

    def din(name, shape, dtype=F16):
        return nc.dram_tensor(name, shape, dtype, kind="ExternalInput")

    x_t = din("x_t", [512, R])                  # [x | ko]^T fp16
    pre_t_d = din("pre_t", [512, R])
    ndemb = din("ndemb", [128, 12 * 128])
    # pre-transposed on host: [p, mo, ki, m] = W[ki*128+p, mo*128+m]
    name_W = din("name_W", [128, 6 * TX])
    desc_W = din("desc_W", [128, 6 * TX])
    omic_W = din("omic_W", [512, 512])
    wnd_W = din("wnd_W", [128, 12 * 512])       # fus_nd @ ienc (host)
    wf_W = din("wf_W", [512, 512])              # fus_om @ ienc (host)
    wpe_W = din("wpe_W", [512, 512])            # pre_W @ enc_W (host)
    enc_W = din("enc_W", [512, 512])
    gate_W1 = din("gate_W1", [512, 512], F16)
    gw2reg = din("gw2reg", [128, 8], F16)
    bias_pf = din("bias_pf", [128, 26], F32)
    bias_rows = din("bias_rows", [96, 512], F16)
    idx1l_d = din("idx1l", [128, (C1L // WAVE) * WCOLS], I16)
    sel1l_d = din("sel1l", [128, C1L * 128], F8)
    idx1r_d = din("idx1r", [128, (C1R // WAVE) * WCOLS], I16)
    sel1r_d = din("sel1r", [128, C1R * 128], F8)
    idx2a_d = din("idx2a", [128, (C2A // WAVE) * WCOLS], I16)
    sel2a_d = din("sel2a", [128, C2A * 128], F16)
    idx2b_d = din("idx2b", [128, (C2B // WAVE) * WCOLS], I16)
    sel2b_d = din("sel2b", [128, C2B * 128], F16)
    out_d = nc.dram_tensor("out", [1, 2], F32, kind="ExternalOutput")

    agnd_in = nc.dram_tensor("agnd_in", [128, 512], F16)
    agnd_out = nc.dram_tensor("agnd_out", [NCORE * 128, 512], F16, addr_space="Shared")
    ag8_in = nc.dram_tensor("ag8_in", [R // 4, 4 * 512], F8)
    ag8_out = nc.dram_tensor("ag8_out", [N // 4, 4 * 512], F8, addr_space="Shared")
    m2_a = nc.dram_tensor("m2_a", [12 * 128, 512], F16)
    m2_b = nc.dram_tensor("m2_b", [4 * 128, 512], F16)
    rs_in = nc.dram_tensor("rs_in", [NCORE * 32, 4 * 512], F16)
    rs_out = nc.dram_tensor("rs_out", [32, 4 * 512], F16)
    RG = [list(range(NCORE))]

    if DEBUG:
        dbg_m2 = nc.dram_tensor("dbg_m2", [R, 512], F16, kind="ExternalOutput")
        dbg_zk = nc.dram_tensor("dbg_zk", [128, 512], F32, kind="ExternalOutput")

    with tile.TileContext(nc) as tc:
        with (
            tc.tile_pool(name="pbig", bufs=8) as pbig,
            tc.tile_pool(name="ppre", bufs=4) as ppre,
            tc.tile_pool(name="pmed", bufs=1) as pmed,
            tc.tile_pool(name="pw", bufs=1) as pw,
            tc.tile_pool(name="pg", bufs=1) as pg,
            tc.tile_pool(name="psc", bufs=1) as psc,
            tc.tile_pool(name="pp", bufs=1, space="PSUM") as pp,
        ):
            # ---- ND embeddings first: one contiguous load feeding the
            # first matmuls ----
            emb_all = psc.tile([128, 12, 128], F16, tag="emb", bufs=1)
            nc.sync.dma_start(
                out=emb_all[:].rearrange("p a c -> p (a c)"), in_=ndemb[:])

            # ---- constants ----
            bpf = psc.tile([128, 26], F32, tag="bpf", bufs=1)
            nc.sync.dma_start(out=bpf[:], in_=bias_pf[:])
            brow_g = psc.tile([1, 512], F16, tag="brow_g", bufs=1)
            nc.sync.dma_start(out=brow_g[:], in_=bias_rows[64:65, :])
            ones = psc.tile([1, 512], F16, tag="ones", bufs=1)
            nc.vector.memset(ones[:], 1.0)
            ident = psc.tile([128, 128], F16, tag="ident", bufs=1)
            make_identity(nc, ident[:])
            idxs = {}
            for nm, dd in (("1l", idx1l_d), ("1r", idx1r_d),
                           ("2a", idx2a_d), ("2b", idx2b_d)):
                t_ = psc.tile([128, dd.shape[1]], I16, tag=f"idx{nm}", bufs=1)
                nc.sync.dma_start(out=t_[:], in_=dd[:])
                idxs[nm] = t_

            # ---- ND path (128 entities) — issued first so AG-nd fires early ----
            nd_act = []
            for half in range(2):
                W_d = name_W if half == 0 else desc_W
                embs = [emb_all[:, 6 * half + ki, :] for ki in range(6)]
                for mo in range(6):
                    ps = pp.tile([128, 512], F32, tag="ps_mm", bufs=2, space="PSUM")
                    wstrip = pw.tile([128, 6, 128], F16, tag="wnd6", bufs=2)
                    nc.sync.dma_start(
                        out=wstrip[:].rearrange("p a m -> p (a m)"),
                        in_=W_d[:, 768 * mo:768 * (mo + 1)])
                    for ki in range(6):
                        nc.tensor.matmul(ps[:, :128], lhsT=wstrip[:, ki, :],
                                         rhs=embs[ki],
                                         start=(ki == 0), stop=(ki == 5))
                    a = psc.tile([128, 128], F16, tag="ndact", bufs=12,
                                 name=f"ndact{half}_{mo}")
                    nc.scalar.activation(a[:], ps[:, :128], ACTF.Prelu,
                                         bias=bpf[:, 6 * half + mo:6 * half + mo + 1],
                                         alpha=SLOPE)
                    nd_act.append(a)
            # nd2 = [name_emb|desc_emb] @ (fus_nd @ ienc_W[:511]) for this
            # core's 128 entities, ROW-major [entity, 512] straight into AG
            wnds = []
            for ki in range(12):
                w_ = pw.tile([128, 512], F16, tag="wnd12", bufs=12)
                nc.sync.dma_start(out=w_[:], in_=wnd_W[:, 512 * ki:512 * (ki + 1)])
                wnds.append(w_)
            ps = pp.tile([128, 512], F32, tag="ps_mm", bufs=2, space="PSUM")
            for ki in range(12):
                nc.tensor.matmul(ps[:], lhsT=nd_act[ki][:], rhs=wnds[ki][:],
                                 start=(ki == 0), stop=(ki == 11))
            r_ = psc.tile([128, 512], F16, tag="ndres", bufs=1)
            nc.scalar.activation(r_[:], ps[:], ACTF.Copy)
            nc.sync.dma_start(out=agnd_in[:], in_=r_[:])
            nc.gpsimd.collective_compute(
                "AllGather", ALU.bypass, replica_groups=RG,
                ins=[agnd_in[:]], outs=[agnd_out[:]])

            # ---- weights first (small; unblock omic/m1/m2d early) ----
            womic = [pw.tile([128, 512], F16, tag="wres", bufs=16, name=f"womic{k}")
                     for k in range(4)]
            wf = [pw.tile([128, 512], F16, tag="wres", bufs=16, name=f"wf{k}")
                  for k in range(4)]
            wenc = [pw.tile([128, 512], F16, tag="wres", bufs=16, name=f"wenc{k}")
                    for k in range(4)]
            wpe = [pw.tile([128, 512], F16, tag="wres", bufs=16, name=f"wpe{k}")
                   for k in range(4)]
            for k in range(4):
                nc.sync.dma_start(out=womic[k][:], in_=omic_W[128 * k:128 * (k + 1), :])
                nc.sync.dma_start(out=wf[k][:], in_=wf_W[128 * k:128 * (k + 1), :])
                nc.sync.dma_start(out=wenc[k][:], in_=enc_W[128 * k:128 * (k + 1), :])
                nc.sync.dma_start(out=wpe[k][:], in_=wpe_W[128 * k:128 * (k + 1), :])
            ienc_ko = psc.tile([1, 512], F16, tag="ienc_ko", bufs=1)
            nc.sync.dma_start(out=ienc_ko[:], in_=bias_rows[65:66, :])
            ko_row = psc.tile([1, R], F16, tag="ko_row", bufs=1)
            nc.sync.dma_start(out=ko_row[:], in_=x_t[511:512, :])

            # ---- big activations (fp16) ----
            xt = []
            for k in range(4):
                t = pbig.tile([128, R], F16, tag="bigA", bufs=4, name=f"xt{k}")
                nc.sync.dma_start(out=t[:], in_=x_t[128 * k:128 * (k + 1), :])
                xt.append(t)
            pre = []
            for k in range(4):
                t = ppre.tile([128, R], F16, tag="pre", bufs=4, name=f"pre{k}")
                nc.sync.dma_start(out=t[:], in_=pre_t_d[128 * k:128 * (k + 1), :])
                pre.append(t)

            # nd2 tiles per entity block, ROW-major. Loaded via the gpsimd
            # (SWDGE) queue: they wait on AG-nd, and on the sync queue that
            # wait would head-of-line block the pre loads.
            nd_sb = [pmed.tile([128, 512], F16, tag="ndsb", bufs=8, name=f"ndsb{e}")
                     for e in range(8)]
            for e in range(8):
                nc.gpsimd.dma_start(out=nd_sb[e][:],
                                    in_=agnd_out[128 * e:128 * (e + 1), :])

            # ---- omic (all j): om_emb feature-major tiles ----
            om_all = {}
            for j in range(4):
                sl = slice(512 * j, 512 * (j + 1))
                for k in range(4):
                    ps = pp.tile([128, 512], F32, tag="ps_mm", bufs=2, space="PSUM")
                    for ki in range(4):
                        nc.tensor.matmul(ps[:], lhsT=womic[ki][:, 128 * k:128 * (k + 1)],
                                         rhs=xt[ki][:, sl], start=(ki == 0), stop=(ki == 3))
                    a = pmed.tile([128, 512], F16, tag="omj", bufs=16)
                    nc.scalar.activation(a[:], ps[:], ACTF.Prelu,
                                         bias=bpf[:, 12 + k:13 + k], alpha=SLOPE)
                    om_all[j, k] = a

            # ---- m1 = om_emb @ (fus_om @ ienc) + nd2[entities] + ko x ienc[511]
            # (cross never materializes; fus and m1 fused via host precompute).
            # uacc[t] is seeded with m1 tile t (the gconv self term) in fp16;
            # ag8_in (fp8) is the only DRAM copy of m1 - local gconv1 edges
            # gather from it during the AllGather.
            uacc = [pmed.tile([128, 512], F16, tag="uacc", bufs=NT, name=f"uacc{t}")
                    for t in range(NT)]
            for t in range(NT):
                tsl = slice(128 * t, 128 * (t + 1))
                j, toff = t // 4, t % 4
                ps = pp.tile([128, 512], F32, tag="ps_seg", bufs=2, space="PSUM")
                for ki in range(4):
                    nc.tensor.matmul(ps[:],
                                     lhsT=om_all[j, ki][:, 128 * toff:128 * (toff + 1)],
                                     rhs=wf[ki][:], start=(ki == 0), stop=False)
                nc.tensor.matmul(ps[:], lhsT=ko_row[:, tsl], rhs=ienc_ko[:],
                                 start=False, stop=False)
                nc.tensor.matmul(ps[:], lhsT=ident[:], rhs=nd_sb[t % 8][:],
                                 start=False, stop=True)
                nc.vector.tensor_copy(out=uacc[t][:], in_=ps[:])
                h8 = pmed.tile([128, 512], F8, tag="m1h8", bufs=2)
                nc.scalar.activation(h8[:], ps[:], ACTF.Copy)
                nc.sync.dma_start(
                    out=ag8_in[32 * t:32 * (t + 1), :].rearrange(
                        "a (b f) -> (a b) f", b=4),
                    in_=h8[:])
            nc.gpsimd.collective_compute(
                "AllGather", ALU.bypass, replica_groups=RG,
                ins=[ag8_in[:]], outs=[ag8_out[:]])

            # ---- generic gather+scatter ----
            gsems = [nc.alloc_semaphore(f"gsem{q}") for q in range(4)]

            def _bounds(nch_t):
                b = []
                for t_id, nch in enumerate(nch_t):
                    for j in range(nch):
                        b.append((t_id, j == 0, j == nch - 1))
                return b

            def scatter(src_dram, idx_t, sel_d, sel_dt, nchunks, tile_bounds,
                        psum_tag, gbufs_n, sfx="", on_a=None, on_b=None,
                        preload_sel=False, sel_sfx=None):
                """Two-phase pipelined per-tile post-processing: on_a(t, ps)
                fires one tile late (at the next tile's first chunk) and
                returns a context; on_b(ctx) fires another tile later. The
                delay keeps the tensor queue from stalling on cross-engine
                dependencies of the post-processing. preload_sel issues every
                sel-wave DMA up front so later sel loads are never head-of-line
                blocked behind result DMAs on the sync queue."""
                out_psums = []
                qa, qb = [], []
                ps = None
                src_ap = src_dram if isinstance(src_dram, bass.AP) else src_dram[:]
                wsz = _wave_sizes(nchunks)
                ssfx = sfx if sel_sfx is None else sel_sfx

                sels = []
                if preload_sel:
                    cur = 0
                    for w, s in enumerate(wsz):
                        sw = pg.tile([128, WAVE, 128], sel_dt, tag="selw" + ssfx,
                                     bufs=len(wsz))
                        nc.sync.dma_start(
                            out=sw[:, :s, :].rearrange("p a d -> p (a d)"),
                            in_=sel_d[:, 128 * cur:128 * (cur + s)])
                        sels.append(sw)
                        cur += s

                def pump():
                    if qb and on_b is not None:
                        on_b(qb.pop(0))
                    if qa:
                        t_id_, ps_ = qa.pop(0)
                        if on_a is not None:
                            ctx = on_a(t_id_, ps_)
                            if on_b is not None:
                                qb.append(ctx)
                cur = 0
                i = 0
                for w, s in enumerate(wsz):
                    g = pg.tile([128, WAVE, 512], sel_dt, tag="gath" + sfx,
                                bufs=gbufs_n)
                    # prepare_only: descriptor gen is the only gpsimd work; the
                    # transfer runs on SDMA after the (cheap) trigger, so waves
                    # on different queues genuinely overlap.
                    q = w % 4
                    if PREP_TRIG:
                        nc.gpsimd.dma_gather(
                            g[:, :s, :], src_ap, idx_t[:, 8 * cur:8 * (cur + s)],
                            s * 128, s * 128, 512,
                            single_packet=True, queue_num=q,
                            prepare_only=True, sem=gsems[q])
                        nc.gpsimd.trigger_dma(count=None, queue_num=q)
                    else:
                        nc.gpsimd.dma_gather(
                            g[:, :s, :], src_ap, idx_t[:, 8 * cur:8 * (cur + s)],
                            s * 128, s * 128, 512,
                            single_packet=True, queue_num=q)
                    if preload_sel:
                        sw = sels[w]
                    else:
                        sw = pg.tile([128, WAVE, 128], sel_dt, tag="selw" + ssfx,
                                     bufs=gbufs_n)
                        nc.sync.dma_start(
                            out=sw[:, :s, :].rearrange("p a d -> p (a d)"),
                            in_=sel_d[:, 128 * cur:128 * (cur + s)])
                    for slot in range(s):
                        t_id, first, last = tile_bounds[i]
                        i += 1
                        if first:
                            pump()
                            ps = pp.tile([128, 512], F32, tag=psum_tag,
                                         bufs=2, space="PSUM")
                        nc.tensor.matmul(ps[:], lhsT=sw[:, slot, :],
                                         rhs=g[:, slot, :],
                                         start=first, stop=last)
                        if last:
                            out_psums.append((t_id, ps))
                            qa.append((t_id, ps))
                    cur += s
                pump()
                pump()
                return out_psums

            # ---- gconv1 local-src edges: gathered fp8 from ag8_in DURING
            # the AllGather. Emitted BEFORE m2d so the tensor queue consumes
            # the 1l waves as they land. ----
            ag8in_rows = ag8_in[:].rearrange("a (b f) -> (a b) f", b=4)
            seg1l = scatter(ag8in_rows, idxs["1l"], sel1l_d, F8, C1L,
                            _bounds(nch1l_t), "ps_seg", 3, sfx="8",
                            sel_sfx="1l8", preload_sel=True)
            for t_id, ps in seg1l:
                nc.vector.tensor_tensor(out=uacc[t_id][:], in0=ps[:],
                                        in1=uacc[t_id][:], op=ALU.add)

            # ---- m2 dense part fills the rest of the AG window:
            #      m2d = x_c @ enc_W + pre_c @ (pre_W @ enc_W) ----
            m2d = [pmed.tile([128, 512], F16, tag="m2d", bufs=NT, name=f"m2d{t}")
                   for t in range(NT)]
            for t in range(NT):
                tsl = slice(128 * t, 128 * (t + 1))
                ps = pp.tile([128, 512], F32, tag="ps_mm", bufs=2, space="PSUM")
                for ki in range(4):
                    nc.tensor.matmul(ps[:], lhsT=xt[ki][:, tsl], rhs=wenc[ki][:],
                                     start=(ki == 0), stop=False)
                for ki in range(4):
                    nc.tensor.matmul(ps[:], lhsT=pre[ki][:, tsl], rhs=wpe[ki][:],
                                     start=False, stop=(ki == 3))
                nc.scalar.activation(m2d[t][:], ps[:], ACTF.Copy)

            # ---- gconv1 remote edges; per tile: u -> lrelu(u)^T strips ->
            #      m2 = m2d + lrelu(u) @ enc_W -> DRAM for gconv2 ----
            ag8_rows = ag8_out[:].rearrange("a (b f) -> (a b) f", b=4)

            def z_a(t_id, ps):
                useg = pmed.tile([128, 512], F16, tag="useg", bufs=2)
                nc.vector.tensor_tensor(out=useg[:], in0=ps[:],
                                        in1=uacc[t_id][:], op=ALU.add)
                pst = pp.tile([128, 512], F16, tag="ps_t", bufs=2, space="PSUM")
                for k in range(4):
                    nc.tensor.transpose(
                        out=pst[:, 128 * k:128 * (k + 1)],
                        in_=useg[:, 128 * k:128 * (k + 1)], identity=ident[:])
                ut = pmed.tile([128, 512], F16, tag="ut", bufs=2)
                for k in range(4):
                    nc.scalar.activation(ut[:, 128 * k:128 * (k + 1)],
                                         pst[:, 128 * k:128 * (k + 1)], ACTF.Prelu,
                                         bias=bpf[:, 16 + k:17 + k], alpha=SLOPE)
                return (t_id, ut)

            def z_b(ctx):
                t_id, ut = ctx
                ps2 = pp.tile([128, 512], F32, tag="ps_m2", bufs=2, space="PSUM")
                for ki in range(4):
                    nc.tensor.matmul(ps2[:], lhsT=ut[:, 128 * ki:128 * (ki + 1)],
                                     rhs=wenc[ki][:], start=(ki == 0), stop=(ki == 3))
                h = pmed.tile([128, 512], F16, tag="m2h", bufs=2)
                nc.vector.tensor_tensor(out=h[:], in0=ps2[:], in1=m2d[t_id][:],
                                        op=ALU.add)
                if DEBUG:
                    nc.sync.dma_start(
                        out=dbg_m2[128 * t_id:128 * (t_id + 1), :], in_=h[:])
                if t_id < 12:
                    nc.sync.dma_start(
                        out=m2_a[128 * t_id:128 * (t_id + 1), :], in_=h[:])
                else:
                    nc.sync.dma_start(
                        out=m2_b[128 * (t_id - 12):128 * (t_id - 11), :], in_=h[:])

            scatter(ag8_rows, idxs["1r"], sel1r_d, F8, C1R,
                    _bounds(nch1r_t), "ps_seg", 3, sfx="8", on_a=z_a, on_b=z_b,
                    preload_sel=True)

            # ---- gconv2: source-side partials over 1024 slots, transposed to
            #      [feat, slot] per slot tile, then ReduceScatter ----
            acc2 = [pmed.tile([128, 512], F16, tag="acc2", bufs=8, name=f"acc2{t}")
                    for t in range(8)]
            seg2a = scatter(m2_a, idxs["2a"], sel2a_d, F16, C2A,
                            _bounds(nch2a_t), "ps_seg", 2)
            for t_id, ps in seg2a:
                nc.vector.tensor_copy(out=acc2[t_id][:], in_=ps[:])

            def rs_tile(t_id, ps):
                pc = pmed.tile([128, 512], F16, tag="m2h", bufs=2)
                nc.vector.tensor_tensor(out=pc[:], in0=ps[:], in1=acc2[t_id][:],
                                        op=ALU.add)
                pst = pp.tile([128, 512], F16, tag="ps_t", bufs=2, space="PSUM")
                for k in range(4):
                    nc.tensor.transpose(
                        out=pst[:, 128 * k:128 * (k + 1)],
                        in_=pc[:, 128 * k:128 * (k + 1)], identity=ident[:])
                zb = pmed.tile([128, 512], F16, tag="zb", bufs=2)
                nc.scalar.activation(zb[:], pst[:], ACTF.Copy)
                nc.sync.dma_start(
                    out=rs_in[32 * t_id:32 * (t_id + 1), :].rearrange(
                        "a (b f) -> (a b) f", b=4),
                    in_=zb[:])

            scatter(m2_b, idxs["2b"], sel2b_d, F16, C2B,
                    _bounds(nch2b_t), "ps_seg", 2, on_a=rs_tile,
                    preload_sel=True, sel_sfx="2b")
            nc.gpsimd.collective_compute(
                "ReduceScatter", ALU.add, replica_groups=RG,
                ins=[rs_in[:]], outs=[rs_out[:]])

            # ---- zk^T straight off the wire + readout ----
            zk16 = pmed.tile([128, 512], F16, tag="zb", bufs=2)
            nc.sync.dma_start(
                out=zk16[:],
                in_=rs_out[:].rearrange("a (b f) -> (a b) f", b=4))
            zkt = pmed.tile([128, 512], F16, tag="zkt", bufs=1)
            for k in range(4):
                kl = slice(128 * k, 128 * (k + 1))
                nc.scalar.activation(zkt[:, kl], zk16[:, kl], ACTF.Prelu,
                                     bias=bpf[:, 20 + k:21 + k], alpha=SLOPE)
            if DEBUG:
                zkf = pmed.tile([128, 512], F32, tag="zkf", bufs=1)
                for k in range(4):
                    ps = pp.tile([128, 512], F16, tag="ps_t", bufs=2, space="PSUM")
                    nc.tensor.transpose(out=ps[:, :128],
                                        in_=zkt[:, 128 * k:128 * (k + 1)],
                                        identity=ident[:])
                    nc.vector.tensor_copy(out=zkf[:, 128 * k:128 * (k + 1)],
                                          in_=ps[:, :128])
                nc.sync.dma_start(out=dbg_zk[:], in_=zkf[:])

            wg1 = [pw.tile([128, 512], F16, tag="wres", bufs=16, name=f"wg1{k}")
                   for k in range(4)]
            for k in range(4):
                nc.sync.dma_start(out=wg1[k][:], in_=gate_W1[128 * k:128 * (k + 1), :])
            w2r = psc.tile([128, 8], F16, tag="w2r", bufs=1)
            nc.sync.dma_start(out=w2r[:], in_=gw2reg[:])
            s1t = pmed.tile([128, 512], F16, tag="s1t", bufs=1)
            for ko_ in range(4):
                ps = pp.tile([128, 512], F32, tag="ps_mm", bufs=2, space="PSUM")
                for ki in range(4):
                    nc.tensor.matmul(ps[:, :128],
                                     lhsT=wg1[ki][:, 128 * ko_:128 * (ko_ + 1)],
                                     rhs=zkt[:, 128 * ki:128 * (ki + 1)],
                                     start=(ki == 0), stop=False)
                nc.tensor.matmul(ps[:, :128],
                                 lhsT=brow_g[:, 128 * ko_:128 * (ko_ + 1)],
                                 rhs=ones[:, :128], start=False, stop=True)
                nc.scalar.activation(s1t[:, 128 * ko_:128 * (ko_ + 1)], ps[:, :128],
                                     ACTF.Tanh)
            ps_sc = pp.tile([128, 512], F32, tag="ps_mm", bufs=2, space="PSUM")
            for ki in range(4):
                nc.tensor.matmul(ps_sc[:1, :128], lhsT=w2r[:, 2 * ki:2 * ki + 1],
                                 rhs=s1t[:, 128 * ki:128 * (ki + 1)],
                                 start=(ki == 0), stop=(ki == 3))
            ps_tr = pp.tile([128, 512], F32, tag="ps_seg", bufs=2, space="PSUM")
            for ki in range(4):
                nc.tensor.matmul(ps_tr[:1, :128], lhsT=w2r[:, 2 * ki + 1:2 * ki + 2],
                                 rhs=zkt[:, 128 * ki:128 * (ki + 1)],
                                 start=(ki == 0), stop=(ki == 3))
            erow = psc.tile([1, 128], F32, tag="erow", bufs=1)
            nc.scalar.activation(erow[:], ps_sc[:1, :128], ACTF.Exp,
                                 bias=bpf[:1, 24:25])
            etrow = psc.tile([1, 128], F32, tag="etrow", bufs=1)
            nc.vector.tensor_tensor(out=etrow[:], in0=erow[:], in1=ps_tr[:1, :128],
                                    op=ALU.mult)
            sums = psc.tile([1, 4], F32, tag="sums", bufs=1)
            nc.vector.tensor_reduce(out=sums[:, 0:2],
                                    in_=etrow[:].rearrange("p (g x) -> p g x", g=2),
                                    axis=AX, op=ALU.add)
            nc.vector.tensor_reduce(out=sums[:, 2:4],
                                    in_=erow[:].rearrange("p (g x) -> p g x", g=2),
                                    axis=AX, op=ALU.add)
            res = psc.tile([1, 4], F32, tag="res", bufs=1)
            nc.vector.reciprocal(out=res[:, 2:4], in_=sums[:, 2:4])
            nc.vector.tensor_tensor(out=res[:, 0:2], in0=sums[:, 0:2],
                                    in1=res[:, 2:4], op=ALU.mult)
            nc.vector.tensor_scalar(out=res[:, 0:2], in0=res[:, 0:2],
                                    scalar1=bpf[:1, 25:26], scalar2=None, op0=ALU.add)
            nc.sync.dma_start(out=out_d[:], in_=res[:, 0:2])

    nc.compile()
    return nc


def _ensure_ntff_hook():
    """Inject antenv.axon_hooks (absent in this image) so trace=True works."""
    import sys, types
    try:
        from antenv.axon_hooks import get_axon_ntff_profile_hook  # noqa
        return
    except ImportError:
        pass
    import antenv
    mod = types.ModuleType("antenv.axon_hooks")
    _state = {"hook": None}
    mod.set_axon_ntff_profile_hook = lambda h: _state.__setitem__("hook", h)
    mod.get_axon_ntff_profile_hook = lambda: _state["hook"]
    sys.modules["antenv.axon_hooks"] = mod
    antenv.axon_hooks = mod
    from trn_agent_boot.trn_boot import _ntff_profile_via_ctypes
    mod.set_axon_ntff_profile_hook(
        _ntff_profile_via_ctypes("/opt/axon/libaxon_pjrt.so"))


# ---------------------------------------------------------------------------
# host wrapper
# ---------------------------------------------------------------------------

def kernel(**inputs):
    f32 = lambda k: np.asarray(inputs[k], np.float32)
    x = f32("x"); pre_x = f32("pre_x")
    edge_index = np.asarray(inputs["edge_index"], np.int64)
    internal_edge_index = np.asarray(inputs["internal_edge_index"], np.int64)
    name_emb = f32("name_embeddings"); desc_emb = f32("desc_embeddings")
    ko_mask = np.asarray(inputs["ko_mask"], np.int64)
    bkm = np.asarray(inputs["batch_ko_masks"], np.int64)
    name_W = f32("name_W"); name_b = f32("name_b")
    desc_W = f32("desc_W"); desc_b = f32("desc_b")
    omic_W = f32("omic_W"); omic_b = f32("omic_b")
    fus_W = f32("fus_W"); fus_b = f32("fus_b")
    pre_W = f32("pre_W"); pre_b = f32("pre_b")
    ienc_W = f32("ienc_W"); ienc_b = f32("ienc_b")
    enc_W = f32("enc_W"); enc_b = f32("enc_b")
    gate_W1 = f32("gate_W1"); gate_b1 = f32("gate_b1")
    gate_W2 = f32("gate_W2"); gate_b2 = f32("gate_b2")
    reg_W = f32("reg_W"); reg_b = f32("reg_b")

    assert not fus_b.any() and not pre_b.any(), \
        "nonzero fus_b/pre_b not supported by this build"

    ko_feat = np.zeros(N, np.float32)
    ko_feat[ko_mask] = 1.0

    # ---- gconv2: source-sharded edges into the 1024 global KO slots ----
    slot_row = (bkm + np.arange(B)[:, None] * NE).reshape(-1)   # [1024]
    row2slots = {}
    for s_, r_ in enumerate(slot_row):
        row2slots.setdefault(int(r_), []).append(s_)
    def _pad_last(nch_t):
        nch_t[-1] += (-int(nch_t.sum())) % WAVE
        return tuple(int(v) for v in nch_t)

    s2_all, d2_all = edge_index[0], edge_index[1]
    m2mask = np.isin(d2_all, slot_row)
    per_core_2a = []   # sources in local rows [0, R/2)
    per_core_2b = []   # sources in local rows [R/2, R)
    needed = []        # per-core local rows whose z/m2 is actually consumed
    nch2a_t = np.ones(8, np.int64)
    nch2b_t = np.ones(8, np.int64)
    for c in range(NCORE):
        lo, hi = R * c, R * (c + 1)
        ss, ds = [], []
        for r_, sl_ in row2slots.items():
            if lo <= r_ < hi:
                for s_ in sl_:
                    ss.append(r_ - lo); ds.append(s_)
        mm = m2mask & (s2_all >= lo) & (s2_all < hi)
        for u, v in zip(s2_all[mm], d2_all[mm]):
            for s_ in row2slots[int(v)]:
                ss.append(int(u) - lo); ds.append(s_)
        src = np.array(ss, np.int64); dstl = np.array(ds, np.int64)
        nd = np.zeros(R, bool)
        nd[src] = True
        needed.append(nd)
        ha = src < 12 * 128
        per_core_2a.append((src[ha], dstl[ha]))
        per_core_2b.append((src[~ha] - 12 * 128, dstl[~ha]))
        nch2a_t = np.maximum(nch2a_t, -(-np.bincount(dstl[ha] >> 7, minlength=8) // 128))
        nch2b_t = np.maximum(nch2b_t, -(-np.bincount(dstl[~ha] >> 7, minlength=8) // 128))
    nch2a_t = _pad_last(nch2a_t)
    nch2b_t = _pad_last(nch2b_t)

    # ---- gconv1 edges (dst-sharded; self term added from local m1h).
    # Edges whose dst row never feeds gconv2 (not a slot row, not a source of
    # a slot edge) are dropped: their z rows are never read. Edges with a
    # LOCAL source are gathered from ag1_in during AG1. ----
    s1_all, d1_all = internal_edge_index[0], internal_edge_index[1]
    per_core_1l = []
    per_core_1r = []
    nch1l_t = np.ones(NT, np.int64)
    nch1r_t = np.ones(NT, np.int64)
    for c in range(NCORE):
        lo, hi = R * c, R * (c + 1)
        m = (d1_all >= lo) & (d1_all < hi)
        s1 = s1_all[m]; d1l = d1_all[m] - lo
        keep = needed[c][d1l]
        s1 = s1[keep]; d1l = d1l[keep]
        isloc = (s1 >= lo) & (s1 < hi)
        # the gconv self term is seeded into uacc from the m1 psum on-chip
        ls = s1[isloc] - lo
        ld = d1l[isloc]
        per_core_1l.append((ls, ld))
        per_core_1r.append((s1[~isloc], d1l[~isloc]))
        nch1l_t = np.maximum(
            nch1l_t, -(-np.bincount(ld >> 7, minlength=NT) // 128))
        nch1r_t = np.maximum(
            nch1r_t, -(-np.bincount(d1l[~isloc] >> 7, minlength=NT) // 128))
    nch1l_t = _pad_last(nch1l_t)
    nch1r_t = _pad_last(nch1r_t)

    nc = _build(nch1l_t, nch1r_t, nch2a_t, nch2b_t, (DEBUG, PREP_TRIG))

    import ml_dtypes
    f16 = np.float16
    f8 = ml_dtypes.float8_e4m3
    omic_Wp = _pad_w(omic_W, 512, 512)
    fus_ndp = _pad_w(fus_W[:2 * TX], 2 * TX, 512)
    fus_omp = _pad_w(fus_W[2 * TX:], 512, 512)
    wpe = pre_W @ enc_W                       # fold z-pre path into m2
    wf = fus_omp[:, :OM] @ ienc_W[:OM, :]     # fuse fus_om into m1
    wnd = fus_ndp[:, :OM] @ ienc_W[:OM, :]    # fuse fus_nd into nd2
    wnd_r = np.ascontiguousarray(
        wnd.reshape(12, 128, 512).transpose(1, 0, 2).reshape(128, 12 * 512))
    # [p, mo, ki, m] = W[ki*128+p, mo*128+m] so wstrip loads are contiguous
    name_Wr = np.ascontiguousarray(
        name_W.reshape(6, 128, 6, 128).transpose(1, 2, 0, 3).reshape(128, 6 * TX))
    desc_Wr = np.ascontiguousarray(
        desc_W.reshape(6, 128, 6, 128).transpose(1, 2, 0, 3).reshape(128, 6 * TX))
    bias_pf = np.zeros((128, 26), np.float32)
    bias_pf[:, 0:6] = name_b.reshape(6, 128).T
    bias_pf[:, 6:12] = desc_b.reshape(6, 128).T
    bias_pf[:, 12:16] = _pad_w(omic_b[:, None], 512, 1).reshape(4, 128).T
    bias_pf[:, 16:20] = ienc_b.reshape(4, 128).T
    bias_pf[:, 20:24] = enc_b.reshape(4, 128).T
    bias_pf[:, 24] = float(gate_b2.reshape(-1)[0])
    bias_pf[:, 25] = float(reg_b.reshape(-1)[0])
    bias_rows = np.zeros((96, 512), np.float32)
    bias_rows[64, :] = gate_b1
    bias_rows[65, :] = ienc_W[OM, :]
    gw2 = np.concatenate([gate_W2, reg_W], axis=1).astype(np.float32)
    gw2 = np.ascontiguousarray(
        gw2.reshape(4, 128, 2).transpose(1, 0, 2).reshape(128, 8))

    shared = dict(
        name_W=name_Wr.astype(f16), desc_W=desc_Wr.astype(f16),
        omic_W=omic_Wp.astype(f16), wnd_W=wnd_r.astype(f16),
        wf_W=wf.astype(f16),
        wpe_W=wpe.astype(f16), enc_W=enc_W.astype(f16),
        gate_W1=gate_W1.astype(f16), gw2reg=gw2.astype(f16), bias_pf=bias_pf,
        bias_rows=bias_rows.astype(f16),
    )

    in_maps = []
    for c in range(NCORE):
        lo, hi = R * c, R * (c + 1)
        x_t = np.concatenate([x[lo:hi].T, ko_feat[None, lo:hi]], 0)
        pre_t = np.concatenate([pre_x[lo:hi].T, ko_feat[None, lo:hi]], 0)
        ndemb = np.concatenate(
            [name_emb[128 * c:128 * (c + 1)].T, desc_emb[128 * c:128 * (c + 1)].T], 0)
        ndemb = ndemb.reshape(12, 128, 128).transpose(1, 0, 2).reshape(128, 12 * 128)
        i1l, dv1l = _chunk_edges_per_tile(*per_core_1l[c], nch1l_t)
        i1r, dv1r = _chunk_edges_per_tile(*per_core_1r[c], nch1r_t)
        i2a, dv2a = _chunk_edges_per_tile(*per_core_2a[c], nch2a_t)
        i2b, dv2b = _chunk_edges_per_tile(*per_core_2b[c], nch2b_t)
        in_maps.append(dict(
            x_t=np.ascontiguousarray(x_t).astype(f16),
            pre_t=np.ascontiguousarray(pre_t).astype(f16),
            ndemb=np.ascontiguousarray(ndemb).astype(f16),
            idx1l=_wrap_idx_waves(i1l), sel1l=_sel_from_dstv(dv1l, f8),
            idx1r=_wrap_idx_waves(i1r), sel1r=_sel_from_dstv(dv1r, f8),
            idx2a=_wrap_idx_waves(i2a), sel2a=_sel_from_dstv(dv2a, f16),
            idx2b=_wrap_idx_waves(i2b), sel2b=_sel_from_dstv(dv2b, f16),
            **shared,
        ))

    if TRACE:
        _ensure_ntff_hook()
    res = run_bass_kernel_spmd(nc, in_maps, core_ids=list(range(NCORE)),
                               trace=TRACE, **(TRACE_KW or {}))
    kernel._last = res
    out = np.zeros(B, np.float32)
    for c in range(NCORE):
        out[2 * c:2 * c + 2] = res.results[c]["out"][0]
    return out
